# revision 12
# baseline (speedup 1.0000x reference)
"""Trainium2 Bass kernel for nn_FDN_88012469830490 (optimized).

FDN reverb: IR synthesis on host (6x6 solves + FFT of the 2x2x88200 IR
into the device spectral layout); FFT convolution on device.

Per core (2 batches x 3 overlap-save blocks of N=262144 = 128*128*16):
  z = x0 + j*x1 channel packing; 3-stage forward FFT in bf16 matmuls with
  the stage-2 twiddle folded into 16 per-n3 stationary matrices; conjugate
  -reversed spectrum via reversed-stride copies with the partition
  permutation folded into the inverse first stage; y0 + j*y1 output
  packing -> single complex inverse FFT per block. Transposes are XBAR DMA
  block-transposes; twiddle/spectral cmuls in bf16 on DVE/GpSimd; psum
  evacuation on ACT/GpSimd/DVE; loads are casting SWDGE DMAs.
"""
import sys
import numpy as np

sys.path.insert(0, "/opt/trn_rl_repo")

import ml_dtypes

BFNP = ml_dtypes.bfloat16

# ---------------- problem constants ----------------
SR = 44100
DELAYS = np.array([997, 1153, 1327, 1559, 1801, 2099])
ND = 6
L = 88200
FB = L // 2 + 1
NDF = 49
T60 = 1.5
GAMMA_MAX = 10.0 ** ((-60.0 / SR / T60 * DELAYS) / 20.0)

T = 441000
N = 262144
P1, P2, P3 = 128, 128, 16
M2 = P2 * P3          # 2048
HOP = N - (L - 1)     # 173945
NBLK = 3
NCORES = 8

VROW, VCOL = 43, 135
ROW_TAIL = 2048 - VCOL            # 1913

NCH = 4
CW = 512


# ---------------- host IR synthesis ----------------
def _expm_skew(S):
    lam, V = np.linalg.eigh(1j * S)
    return (V @ np.diag(np.exp(-1j * lam)) @ V.conj().T).real


def _host_ir(b, c, U_raw, gamma_raw):
    tri = np.triu(U_raw.astype(np.float64), 1)
    U = _expm_skew(tri - tri.T)
    gamma = (1.0 / (1.0 + np.exp(-gamma_raw.astype(np.float64)))) * GAMMA_MAX
    pos = np.arange(FB) * ((NDF - 1) / (FB - 1))
    i0 = np.clip(np.floor(pos).astype(int), 0, NDF - 2)
    frac = (pos - i0)[:, None]
    g = gamma[i0] * (1 - frac) + gamma[i0 + 1] * frac
    A = U[None, :, :] * g[:, None, :]
    freqs = np.arange(FB) / L * 2 * np.pi
    invD = np.exp(1j * freqs[:, None] * DELAYS)
    Mm = invD[:, :, None] * np.eye(ND) - A
    bc = np.broadcast_to(b.astype(np.float64), (FB, ND, 2))
    X = np.linalg.solve(Mm, bc)
    H = np.einsum('ci,fio->fco', c.astype(complex), X)
    h = np.fft.irfft(H.transpose(1, 2, 0), n=L)             # (2, 2, L)
    return h


# ---------------- spectral layout + tables ----------------
def _kmap():
    P = np.arange(128)[:, None]
    F = np.arange(M2)[None, :]
    k1 = (P >> 4) + 8 * (F >> 7)
    return k1 + 128 * (F & 127) + 16384 * (P & 15)


KMAP = _kmap()
PPERM = np.array([((8 - (p >> 4)) % 8) * 16 + (15 - (p & 15)) for p in range(128)])


def _m3(M):
    """lhsT triple (Mr.T, Mi.T, -Mi.T) for out = M @ rhs (standard)."""
    Mt = M.T
    return [np.ascontiguousarray(Mt.real, np.float32),
            np.ascontiguousarray(Mt.imag, np.float32),
            np.ascontiguousarray(-Mt.imag, np.float32)]


def _m3c(M):
    """lhsT triple (Mr.T, Mi.T, -Mr.T) for out = M @ conj(rhs)."""
    Mt = M.T
    return [np.ascontiguousarray(Mt.real, np.float32),
            np.ascontiguousarray(Mt.imag, np.float32),
            np.ascontiguousarray(-Mt.real, np.float32)]


def _host_tables():
    F128 = np.exp(-2j * np.pi * np.outer(np.arange(128), np.arange(128)) / 128)
    F16 = np.exp(-2j * np.pi * np.outer(np.arange(16), np.arange(16)) / 16)
    Bc = np.zeros((128, 128), complex)
    for kk in range(8):
        Bc[kk * 16:(kk + 1) * 16, kk * 16:(kk + 1) * 16] = F16

    mats = []
    IDX = {}

    def push(tr, key=None):
        if key is not None:
            IDX[key] = len(mats)
        mats.extend(tr)

    push(_m3(F128), "s1")
    IDX["s2f"] = len(mats)
    for n3 in range(16):
        tw = np.exp(-2j * np.pi * np.arange(128) * n3 / M2)
        push(_m3(F128 * tw[:, None]))
    push(_m3(Bc), "s3")
    BH = np.conj(Bc)                   # = conj(Bc).T, Bc block-symmetric
    push(_m3(BH), "i1a")
    push(_m3c(BH[:, PPERM]), "i1b")
    IDX["s2i"] = len(mats)
    for n3 in range(16):
        tw = np.exp(-2j * np.pi * np.arange(128) * n3 / M2)
        M = (np.conj(F128) * np.conj(tw)[:, None]).T   # [n2, q1]
        push(_m3(M))
    push(_m3(np.conj(F128) / N), "s3i")
    mats_f32 = np.stack(mats)

    # wide twiddle tables, n3-major layouts
    k1c = np.arange(128)[:, None]
    n3g = np.arange(M2)[None, :] >> 7
    ag = np.arange(M2)[None, :] & 127
    t1 = np.exp(-2j * np.pi * (k1c * (ag * 16 + n3g)) / N)      # [k1, n3*128+a]
    n2c = np.arange(128)[:, None]
    k1g = np.arange(M2)[None, :] & 127
    it1 = np.conj(np.exp(-2j * np.pi * (k1g * (n2c * 16 + n3g)) / N))
    tabs = np.stack([t1.real, t1.imag, it1.real, it1.imag]).astype(np.float32)
    return IDX, mats_f32, tabs


_IDX, _MATS_F32, _TABS_F32 = _host_tables()
_NMATS = _MATS_F32.shape[0]


def _pq_tables(h):
    hp = np.zeros((2, 2, N))
    hp[:, :, :L] = h
    Hf = np.fft.fft(hp, axis=-1)
    G0 = Hf[0, 0] + 1j * Hf[1, 0]
    G1 = Hf[0, 1] + 1j * Hf[1, 1]
    Pt = (G0 - 1j * G1) / 2
    Qt = (G0 + 1j * G1) / 2
    Play = Pt[KMAP]
    Qtil = np.conj(Qt[KMAP][PPERM, :])
    return np.stack([Play.real, Play.imag, Qtil.real, Qtil.imag]).astype(np.float32)


# ---------------- bass program ----------------
_PROG = None


def _build_program():
    import concourse.bass as bass
    import concourse.tile as tile
    from concourse import bacc, mybir
    from contextlib import ExitStack

    f32 = mybir.dt.float32
    bf16 = mybir.dt.bfloat16
    nc = bacc.Bacc("TRN2", target_bir_lowering=False, debug=False,
                   enable_asserts=False, num_devices=NCORES)

    xp = nc.dram_tensor("xp", [2, 2, T], f32, kind="ExternalInput").ap()
    mats_d = nc.dram_tensor("mats", [_NMATS, 128, 128], bf16, kind="ExternalInput").ap()
    tabs_d = nc.dram_tensor("tabs", [4, 128, M2], bf16, kind="ExternalInput").ap()
    pq_d = nc.dram_tensor("pq", [4, 128, M2], bf16, kind="ExternalInput").ap()
    yp = nc.dram_tensor("yp", [2, 2, T], f32, kind="ExternalOutput").ap()

    with tile.TileContext(nc) as tc, ExitStack() as ctx:
        cpool = ctx.enter_context(tc.tile_pool(name="consts", bufs=1))
        work = ctx.enter_context(tc.tile_pool(name="work", bufs=1))
        psA = ctx.enter_context(tc.tile_pool(name="psA", bufs=2, space="PSUM"))
        psB = ctx.enter_context(tc.tile_pool(name="psB", bufs=2, space="PSUM"))

        # ---- constants ----
        matst = cpool.tile([128, _NMATS * 128], bf16, tag="mats")
        tabst = cpool.tile([128, 4 * M2], bf16, tag="tabs")
        pqt = cpool.tile([128, 4 * M2], bf16, tag="pq")

        def _load_mats(eng, k0, k1):
            eng.dma_start(
                matst[:, k0 * 128:k1 * 128].rearrange("p (k c) -> p k c", k=k1 - k0),
                mats_d[k0:k1, :, :].rearrange("k p c -> p k c"))

        def _load_tab(eng, dstt, srct, k0, k1):
            eng.dma_start(
                dstt[:, k0 * M2:k1 * M2].rearrange("p (k c) -> p k c", k=k1 - k0),
                srct[k0:k1, :, :].rearrange("k p c -> p k c"))

        # s1 mats + t1 tabs now; the rest deferred into the schedule
        _load_mats(nc.sync, _IDX["s1"], _IDX["s1"] + 3)
        _load_tab(nc.scalar, tabst, tabs_d, 0, 2)          # t1
        deferred_loads = [
            (0.4, lambda: _load_mats(nc.sync, _IDX["s2f"], _IDX["s2f"] + 48)),
            (0.5, lambda: _load_mats(nc.scalar, _IDX["s3"], _IDX["s3"] + 3)),
            (4.5, lambda: _load_tab(nc.sync, pqt, pq_d, 0, 4)),
            (5.5, lambda: _load_mats(nc.scalar, _IDX["i1a"], _IDX["i1a"] + 6)),
            (6.5, lambda: _load_tab(nc.sync, tabst, tabs_d, 2, 4)),
            (7.5, lambda: _load_mats(nc.scalar, _IDX["s2i"], _IDX["s2i"] + 48)),
            (8.5, lambda: _load_mats(nc.sync, _IDX["s3i"], _IDX["s3i"] + 3)),
        ]

        def mat(i):
            return matst[:, i * 128:(i + 1) * 128]

        def m3(key, off=0):
            base = _IDX[key] + 3 * off
            return mat(base), mat(base + 1), mat(base + 2)

        def tab(i):
            return tabst[:, i * M2:(i + 1) * M2]

        t1r, t1i = tab(0), tab(1)
        it1r, it1i = tab(2), tab(3)
        Pr = pqt[:, 0:M2]
        Pi = pqt[:, M2:2 * M2]
        Qr = pqt[:, 2 * M2:3 * M2]
        Qi = pqt[:, 3 * M2:4 * M2]

        s1m = m3("s1")
        s2fm = [m3("s2f", n3) for n3 in range(16)]
        s3m = m3("s3")
        i1am = m3("i1a")
        i1bm = m3("i1b")
        s2im = [m3("s2i", n3) for n3 in range(16)]
        s3im = m3("s3i")

        # ---- helpers ----
        def _copy(eng, dst, src):
            if eng is nc.scalar:
                nc.scalar.copy(dst, src)
            else:
                eng.tensor_copy(dst, src)

        def copy_chunk(eng, dst, src_ps, ch, rearr):
            if rearr is None:
                _copy(eng, dst[:, ch * CW:(ch + 1) * CW], src_ps[:])
            elif rearr == "m2n":
                # psum cols m-local = (a-32ch)*16+n3 ; dst col n3*128+a
                psrc = src_ps[:].rearrange("p (a n3) -> p a n3", n3=16)
                ddst = dst[:].rearrange("p (n3 a) -> p a n3", n3=16)[
                    :, ch * 32:(ch + 1) * 32, :]
                _copy(eng, ddst, psrc)
            elif rearr == "k2n":
                psrc = src_ps[:].rearrange("p (k n3) -> p k n3", n3=16)
                ddst = dst[:].rearrange("p (n3 k) -> p k n3", n3=16)[
                    :, ch * 32:(ch + 1) * 32, :]
                _copy(eng, ddst, psrc)
            elif rearr == "n2m":
                # psum cols = (n3-4ch)*128+n2 ; dst col m = n2*16+n3
                psrc = src_ps[:].rearrange("p (n3 n2) -> p n2 n3", n3=4)
                ddst = dst[:].rearrange("p (n2 n3) -> p n2 n3", n3=16)[
                    :, :, ch * 4:(ch + 1) * 4]
                _copy(eng, ddst, psrc)

        def cstage_full(rhs_r, rhs_i, m3_, pspool, post):
            mr, mi, nmi = m3_
            for ch in range(NCH):
                sl = (slice(None), slice(ch * CW, (ch + 1) * CW))
                prr = pspool.tile([128, CW], f32, tag="pr")
                pii = pspool.tile([128, CW], f32, tag="pi")
                nc.tensor.matmul(prr[:], mr[:], rhs_r[sl], start=True, stop=False)
                nc.tensor.matmul(prr[:], nmi[:], rhs_i[sl], start=False, stop=True)
                nc.tensor.matmul(pii[:], mi[:], rhs_r[sl], start=True, stop=False)
                nc.tensor.matmul(pii[:], mr[:], rhs_i[sl], start=False, stop=True)
                post(prr, pii, ch)

        def s2_stage(rhs_r, rhs_i, mlist, pspool, post, rhs_kmajor):
            for ch in range(NCH):
                prr = pspool.tile([128, CW], f32, tag="pr")
                pii = pspool.tile([128, CW], f32, tag="pi")
                k0 = ch * 32
                for n3 in range(16):
                    mr, mi, nmi = mlist[n3]
                    osl = (slice(None), slice(n3, CW, 16))
                    if rhs_kmajor:
                        rsl = (slice(None),
                               slice(k0 * 16 + n3, (k0 + 31) * 16 + n3 + 1, 16))
                    else:
                        rsl = (slice(None),
                               slice(n3 * 128 + k0, n3 * 128 + k0 + 32))
                    nc.tensor.matmul(prr[osl], mr[:], rhs_r[rsl], start=True, stop=False)
                    nc.tensor.matmul(prr[osl], nmi[:], rhs_i[rsl], start=False, stop=True)
                    nc.tensor.matmul(pii[osl], mi[:], rhs_r[rsl], start=True, stop=False)
                    nc.tensor.matmul(pii[osl], mr[:], rhs_i[rsl], start=False, stop=True)
                post(prr, pii, ch)

        def cmul_chunk(dst_r, dst_i, ar, ai, br, bi, ch, lay,
                       mule1, mule2, addeng1, addeng2):
            """One CW-chunk of (dst_r + j dst_i) = (ar + j ai)*(br + j bi).
            lay: "c" contiguous chunk; "n3a" 3D region [p, n3(16), 32a]."""
            def reg(t_):
                if lay == "c":
                    return t_[:, ch * CW:(ch + 1) * CW]
                return t_[:].rearrange("p (n3 a) -> p n3 a", n3=16)[
                    :, :, ch * 32:(ch + 1) * 32]

            m0 = work.tile([128, CW], bf16, tag="cm0", bufs=6, name=f"m0c")
            m1 = work.tile([128, CW], bf16, tag="cm1", bufs=6, name=f"m1c")
            m2 = work.tile([128, CW], bf16, tag="cm2", bufs=6, name=f"m2c")
            m3_ = work.tile([128, CW], bf16, tag="cm3", bufs=6, name=f"m3c")
            arr, aii = reg(ar), reg(ai)
            brr, bii = reg(br), reg(bi)
            mule1.tensor_mul(m0[:].rearrange("p (n3 a) -> p n3 a", n3=16) if lay == "n3a" else m0[:], arr, brr)
            mule1.tensor_mul(m1[:].rearrange("p (n3 a) -> p n3 a", n3=16) if lay == "n3a" else m1[:], aii, bii)
            mule2.tensor_mul(m2[:].rearrange("p (n3 a) -> p n3 a", n3=16) if lay == "n3a" else m2[:], arr, bii)
            mule2.tensor_mul(m3_[:].rearrange("p (n3 a) -> p n3 a", n3=16) if lay == "n3a" else m3_[:], aii, brr)
            sm0 = m0[:].rearrange("p (n3 a) -> p n3 a", n3=16) if lay == "n3a" else m0[:]
            sm1 = m1[:].rearrange("p (n3 a) -> p n3 a", n3=16) if lay == "n3a" else m1[:]
            sm2 = m2[:].rearrange("p (n3 a) -> p n3 a", n3=16) if lay == "n3a" else m2[:]
            sm3 = m3_[:].rearrange("p (n3 a) -> p n3 a", n3=16) if lay == "n3a" else m3_[:]
            addeng1.tensor_sub(reg(dst_r), sm0, sm1)
            addeng2.tensor_add(reg(dst_i), sm2, sm3)

        def xbar_T(dst, src, eng):
            eng.dma_start_transpose(
                dst[:].rearrange("j (g p) -> j g p", p=128), src[:])

        # ---- load / store ----
        def load_block(in_r, in_i, b, blk):
            """Load x into bf16 tiles; only the nonzero partition range is
            populated (S1 contracts over ROWRNG[blk]); tiny edge memsets."""
            for pl, t_ in ((0, in_r), (1, in_i)):
                src = xp[b, pl]
                if blk == 0:
                    nc.vector.memset(t_[0:VROW + 1, :], 0.0)
                    nc.gpsimd.dma_start(
                        t_[VROW:VROW + 1, VCOL:M2],
                        src[0:ROW_TAIL].rearrange('(a b) -> a b', a=1))
                    nc.gpsimd.dma_start(
                        t_[VROW + 1:128, :],
                        src[ROW_TAIL:HOP].rearrange("(r m) -> r m", m=M2))
                elif blk == 1:
                    s0 = HOP - (L - 1)
                    nc.gpsimd.dma_start(
                        t_[:, :], src[s0:s0 + N].rearrange("(r m) -> r m", m=M2))
                else:
                    s0 = 2 * HOP - (L - 1)
                    nfull = (T - s0) // M2
                    rem = (T - s0) - nfull * M2
                    nc.vector.memset(t_[64:128, rem:M2], 0.0)
                    nc.gpsimd.dma_start(
                        t_[0:nfull, :],
                        src[s0:s0 + nfull * M2].rearrange("(r m) -> r m", m=M2))
                    nc.gpsimd.dma_start(
                        t_[nfull:nfull + 1, 0:rem],
                        src[s0 + nfull * M2:T].rearrange('(a b) -> a b', a=1))

        ROWRNG = {0: (0, 128), 1: (0, 128), 2: (0, 89)}

        def store_block(ysb0, ysb1, b, blk):
            for o, ysb in ((0, ysb0), (1, ysb1)):
                dst = yp[b, o]
                base = blk * HOP
                eng = nc.sync if o == 0 else nc.scalar
                eng.dma_start(
                    dst[base:base + ROW_TAIL].rearrange('(a b) -> a b', a=1),
                    ysb[VROW:VROW + 1, VCOL:M2])
                if blk < 2:
                    eng.dma_start(
                        dst[base + ROW_TAIL:base + HOP].rearrange("(r m) -> r m", m=M2),
                        ysb[VROW + 1:128, :])
                else:
                    nrem = T - base - ROW_TAIL
                    nfull = nrem // M2
                    rem = nrem - nfull * M2
                    eng.dma_start(
                        dst[base + ROW_TAIL:base + ROW_TAIL + nfull * M2]
                            .rearrange("(r m) -> r m", m=M2),
                        ysb[VROW + 1:VROW + 1 + nfull, :])
                    eng.dma_start(
                        dst[T - rem:T].rearrange('(a b) -> a b', a=1),
                        ysb[VROW + 1 + nfull:VROW + 2 + nfull, 0:rem])

        # ---- per-(batch, block) pipeline, stage closures ----
        def make_stages(b, blk, js):
            """Return list of stage closures for block (b, blk) using tag
            suffix js (job slot). Tiles are pre-created here; slot reuse:
            A: in->d2->z->sb->g2 ; B: g0->z2->w->s1c->h ; C: g->d3->sa->dp->d4."""
            sfx = f"_{js}"

            ctr = [0]

            def _slot(sl):
                ctr[0] += 1
                return (work.tile([128, M2], bf16, tag=sl + "r" + sfx,
                                  name=f"{sl}r{sfx}_{ctr[0]}"),
                        work.tile([128, M2], bf16, tag=sl + "i" + sfx,
                                  name=f"{sl}i{sfx}_{ctr[0]}"))

            def tA():
                return _slot("A")

            def tB():
                return _slot("B")

            def tC():
                return _slot("C")

            in_r, in_i = tA()
            g0r, g0i = tB()
            gr, gi = tC()
            d2r, d2i = tA()
            z2r, z2i = tB()
            d3r, d3i = tC()
            zr, zi = tA()
            wr_, wi_ = tB()
            sar, sai = tC()
            sbr, sbi = tA()
            s1cr, s1ci = tB()
            dpr, dpi = tC()
            g2r, g2i = tA()
            hr, hi = tB()
            d4r, d4i = tC()
            ysb0 = work.tile([128, M2], f32, tag="ysb0" + sfx, name="ysb0" + sfx + str(blk))
            ysb1 = work.tile([128, M2], f32, tag="ysb1" + sfx, name="ysb1" + sfx + str(blk))

            st = []
            st.append(lambda: load_block(in_r, in_i, b, blk))

            def f_s1_t1():
                mr, mi, nmi = s1m
                r0, r1 = ROWRNG[blk]
                for ch in range(NCH):
                    sl = (slice(r0, r1), slice(ch * CW, (ch + 1) * CW))
                    msl = slice(r0, r1)
                    prr = psA.tile([128, CW], f32, tag="pr")
                    pii = psA.tile([128, CW], f32, tag="pi")
                    nc.tensor.matmul(prr[:], mr[msl], in_r[sl], start=True, stop=False)
                    nc.tensor.matmul(prr[:], nmi[msl], in_i[sl], start=False, stop=True)
                    nc.tensor.matmul(pii[:], mi[msl], in_r[sl], start=True, stop=False)
                    nc.tensor.matmul(pii[:], mr[msl], in_i[sl], start=False, stop=True)
                    copy_chunk(nc.scalar, g0r, prr, ch, "m2n")
                    copy_chunk(nc.scalar, g0i, pii, ch, "m2n")
                    cmul_chunk(gr, gi, g0r, g0i, t1r, t1i, ch, "n3a",
                               nc.vector, nc.vector, nc.gpsimd, nc.vector)
            st.append(f_s1_t1)
            st.append(lambda: (xbar_T(d2r, gr, nc.sync),
                               xbar_T(d2i, gi, nc.sync)))
            st.append(lambda: s2_stage(
                d2r, d2i, s2fm, psA,
                lambda pr, pi, ch: (copy_chunk(nc.scalar, z2r, pr, ch, None),
                                    copy_chunk(nc.scalar, z2i, pi, ch, None)),
                rhs_kmajor=False))
            st.append(lambda: (xbar_T(d3r, z2r, nc.scalar),
                               xbar_T(d3i, z2i, nc.scalar)))
            st.append(lambda: cstage_full(
                d3r, d3i, s3m, psB,
                lambda pr, pi, ch: (copy_chunk(nc.scalar, zr, pr, ch, None),
                                    copy_chunk(nc.scalar, zi, pi, ch, None))))

            def w_stage():
                for zsrc, wdst in ((zr, wr_), (zi, wi_)):
                    nc.vector.tensor_copy(wdst[0:128, 0:M2], zsrc[0:128, M2 - 1::-1])
                    nc.vector.tensor_copy(wdst[0:16, 128:M2], zsrc[0:16, M2 - 1:127:-1])
                    nc.vector.tensor_copy(wdst[0:16, 1:128], zsrc[0:16, 127:0:-1])
                    nc.scalar.dma_start(wdst[0:15, 0:1], zsrc[1:16, 0:1])
                    nc.scalar.dma_start(wdst[15:16, 0:1], zsrc[0:1, 0:1])
            st.append(w_stage)

            def f_spec_s1inv():
                a0, a1, a2 = i1am
                b0, b1, b2 = i1bm
                for ch in range(NCH):
                    cmul_chunk(sar, sai, zr, zi, Pr, Pi, ch, "c",
                               nc.vector, nc.vector, nc.vector, nc.vector)
                    cmul_chunk(sbr, sbi, wr_, wi_, Qr, Qi, ch, "c",
                               nc.vector, nc.vector, nc.gpsimd, nc.gpsimd)
                    sl = (slice(None), slice(ch * CW, (ch + 1) * CW))
                    prr = psB.tile([128, CW], f32, tag="pr")
                    pii = psB.tile([128, CW], f32, tag="pi")
                    nc.tensor.matmul(prr[:], a0[:], sar[sl], start=True, stop=False)
                    nc.tensor.matmul(prr[:], a2[:], sai[sl], start=False, stop=False)
                    nc.tensor.matmul(prr[:], b0[:], sbr[sl], start=False, stop=False)
                    nc.tensor.matmul(prr[:], b1[:], sbi[sl], start=False, stop=True)
                    nc.tensor.matmul(pii[:], a1[:], sar[sl], start=True, stop=False)
                    nc.tensor.matmul(pii[:], a0[:], sai[sl], start=False, stop=False)
                    nc.tensor.matmul(pii[:], b1[:], sbr[sl], start=False, stop=False)
                    nc.tensor.matmul(pii[:], b2[:], sbi[sl], start=False, stop=True)
                    copy_chunk(nc.scalar, s1cr, prr, ch, None)
                    copy_chunk(nc.scalar, s1ci, pii, ch, None)
            st.append(f_spec_s1inv)
            st.append(lambda: (xbar_T(dpr, s1cr, nc.sync),
                               xbar_T(dpi, s1ci, nc.sync)))

            def f_s2i_it1():
                for ch in range(NCH):
                    prr = psB.tile([128, CW], f32, tag="pr")
                    pii = psB.tile([128, CW], f32, tag="pi")
                    k0 = ch * 32
                    for n3 in range(16):
                        mr, mi, nmi = s2im[n3]
                        osl = (slice(None), slice(n3, CW, 16))
                        rsl = (slice(None),
                               slice(k0 * 16 + n3, (k0 + 31) * 16 + n3 + 1, 16))
                        nc.tensor.matmul(prr[osl], mr[:], dpr[rsl], start=True, stop=False)
                        nc.tensor.matmul(prr[osl], nmi[:], dpi[rsl], start=False, stop=True)
                        nc.tensor.matmul(pii[osl], mi[:], dpr[rsl], start=True, stop=False)
                        nc.tensor.matmul(pii[osl], mr[:], dpi[rsl], start=False, stop=True)
                    copy_chunk(nc.scalar, g2r, prr, ch, "k2n")
                    copy_chunk(nc.scalar, g2i, pii, ch, "k2n")
                    cmul_chunk(hr, hi, g2r, g2i, it1r, it1i, ch, "n3a",
                               nc.vector, nc.vector, nc.gpsimd, nc.vector)
            st.append(f_s2i_it1)
            st.append(lambda: (xbar_T(d4r, hr, nc.sync),
                               xbar_T(d4i, hi, nc.sync)))
            st.append(lambda: cstage_full(
                d4r, d4i, s3im, psA,
                lambda pr, pi, ch: (copy_chunk(nc.scalar, ysb0, pr, ch, "n2m"),
                                    copy_chunk(nc.scalar, ysb1, pi, ch, "n2m"))))
            st.append(lambda: store_block(ysb0, ysb1, b, blk))

            return st

        jobs = [(0, 0), (1, 0), (0, 1), (1, 1), (0, 2), (1, 2)]
        WAVE2_OFF = 14
        chains = [make_stages(b_, blk_, j % 3) for j, (b_, blk_) in enumerate(jobs)]
        sched = []
        for j, ch in enumerate(chains):
            off = 0 if j < 3 else WAVE2_OFF
            for s, fn in enumerate(ch):
                sched.append((off + s, 10 + j, fn))
        for r, fn in deferred_loads:
            sched.append((r, 0, fn))
        sched.sort(key=lambda t: (t[0], t[1]))
        for _, _, fn in sched:
            fn()

    nc.compile()
    return nc


def _get_prog():
    global _PROG
    if _PROG is None:
        _PROG = _build_program()
    return _PROG


# ---------------- public entry ----------------
def kernel(x, b, c, U_raw, gamma_raw):
    from concourse import bass_utils

    x = np.ascontiguousarray(np.asarray(x, np.float32))
    h = _host_ir(np.asarray(b, np.float32), np.asarray(c, np.float32),
                 np.asarray(U_raw, np.float32), np.asarray(gamma_raw, np.float32))
    pqf = _pq_tables(h)
    mats_bf = _MATS_F32.astype(BFNP)
    tabs_bf = _TABS_F32.astype(BFNP)
    pq_bf = pqf.astype(BFNP)
    nc = _get_prog()

    in_maps = []
    for core in range(NCORES):
        in_maps.append({
            "xp": x[2 * core:2 * core + 2],
            "mats": mats_bf, "tabs": tabs_bf, "pq": pq_bf,
        })
    res = bass_utils.run_bass_kernel_spmd(nc, in_maps, core_ids=list(range(NCORES)))
    y = np.empty((16, 2, T), np.float32)
    for core in range(NCORES):
        y[2 * core:2 * core + 2] = res.results[core]["yp"]
    return y


# revision 18
# speedup vs baseline: 1.0037x; 1.0037x over previous
"""Trainium2 Bass kernel for nn_FDN_88012469830490 (optimized).

FDN reverb: IR synthesis on host (6x6 solves + FFT of the 2x2x88200 IR
into the device spectral layout); FFT convolution on device.

Per core (2 batches x 3 overlap-save blocks of N=262144 = 128*128*16):
  z = x0 + j*x1 channel packing; 3-stage forward FFT in bf16 matmuls with
  the stage-2 twiddle folded into 16 per-n3 stationary matrices; conjugate
  -reversed spectrum via reversed-stride copies with the partition
  permutation folded into the inverse first stage; y0 + j*y1 output
  packing -> single complex inverse FFT per block. Transposes are XBAR DMA
  block-transposes; twiddle/spectral cmuls in bf16 on DVE/GpSimd; psum
  evacuation on ACT/GpSimd/DVE; loads are casting SWDGE DMAs.
"""
import sys
import numpy as np

sys.path.insert(0, "/opt/trn_rl_repo")

import ml_dtypes

BFNP = ml_dtypes.bfloat16

# ---------------- problem constants ----------------
SR = 44100
DELAYS = np.array([997, 1153, 1327, 1559, 1801, 2099])
ND = 6
L = 88200
FB = L // 2 + 1
NDF = 49
T60 = 1.5
GAMMA_MAX = 10.0 ** ((-60.0 / SR / T60 * DELAYS) / 20.0)

T = 441000
N = 262144
P1, P2, P3 = 128, 128, 16
M2 = P2 * P3          # 2048
HOP = N - (L - 1)     # 173945
NBLK = 3
NCORES = 8

VROW, VCOL = 43, 135
ROW_TAIL = 2048 - VCOL            # 1913

NCH = 4
CW = 512


# ---------------- host IR synthesis ----------------
def _expm_skew(S):
    lam, V = np.linalg.eigh(1j * S)
    return (V @ np.diag(np.exp(-1j * lam)) @ V.conj().T).real


def _host_ir(b, c, U_raw, gamma_raw):
    tri = np.triu(U_raw.astype(np.float64), 1)
    U = _expm_skew(tri - tri.T)
    gamma = (1.0 / (1.0 + np.exp(-gamma_raw.astype(np.float64)))) * GAMMA_MAX
    pos = np.arange(FB) * ((NDF - 1) / (FB - 1))
    i0 = np.clip(np.floor(pos).astype(int), 0, NDF - 2)
    frac = (pos - i0)[:, None]
    g = gamma[i0] * (1 - frac) + gamma[i0 + 1] * frac
    A = U[None, :, :] * g[:, None, :]
    freqs = np.arange(FB) / L * 2 * np.pi
    invD = np.exp(1j * freqs[:, None] * DELAYS)
    Mm = invD[:, :, None] * np.eye(ND) - A
    bc = np.broadcast_to(b.astype(np.float64), (FB, ND, 2))
    X = np.linalg.solve(Mm, bc)
    H = np.einsum('ci,fio->fco', c.astype(complex), X)
    h = np.fft.irfft(H.transpose(1, 2, 0), n=L)             # (2, 2, L)
    return h


# ---------------- spectral layout + tables ----------------
def _kmap():
    P = np.arange(128)[:, None]
    F = np.arange(M2)[None, :]
    k1 = (P >> 4) + 8 * (F >> 7)
    return k1 + 128 * (F & 127) + 16384 * (P & 15)


KMAP = _kmap()
PPERM = np.array([((8 - (p >> 4)) % 8) * 16 + (15 - (p & 15)) for p in range(128)])


def _m3(M):
    """lhsT triple (Mr.T, Mi.T, -Mi.T) for out = M @ rhs (standard)."""
    Mt = M.T
    return [np.ascontiguousarray(Mt.real, np.float32),
            np.ascontiguousarray(Mt.imag, np.float32),
            np.ascontiguousarray(-Mt.imag, np.float32)]


def _m3c(M):
    """lhsT triple (Mr.T, Mi.T, -Mr.T) for out = M @ conj(rhs)."""
    Mt = M.T
    return [np.ascontiguousarray(Mt.real, np.float32),
            np.ascontiguousarray(Mt.imag, np.float32),
            np.ascontiguousarray(-Mt.real, np.float32)]


def _host_tables():
    F128 = np.exp(-2j * np.pi * np.outer(np.arange(128), np.arange(128)) / 128)
    F16 = np.exp(-2j * np.pi * np.outer(np.arange(16), np.arange(16)) / 16)
    Bc = np.zeros((128, 128), complex)
    for kk in range(8):
        Bc[kk * 16:(kk + 1) * 16, kk * 16:(kk + 1) * 16] = F16

    mats = []
    IDX = {}

    def push(tr, key=None):
        if key is not None:
            IDX[key] = len(mats)
        mats.extend(tr)

    push(_m3(F128), "s1")
    IDX["s2f"] = len(mats)
    for n3 in range(16):
        tw = np.exp(-2j * np.pi * np.arange(128) * n3 / M2)
        push(_m3(F128 * tw[:, None]))
    push(_m3(Bc), "s3")
    BH = np.conj(Bc)                   # = conj(Bc).T, Bc block-symmetric
    push(_m3(BH), "i1a")
    push(_m3c(BH[:, PPERM]), "i1b")
    IDX["s2i"] = len(mats)
    for n3 in range(16):
        tw = np.exp(-2j * np.pi * np.arange(128) * n3 / M2)
        M = (np.conj(F128) * np.conj(tw)[:, None]).T   # [n2, q1]
        push(_m3(M))
    push(_m3(np.conj(F128) / N), "s3i")
    mats_f32 = np.stack(mats)

    # wide twiddle tables, n3-major layouts
    k1c = np.arange(128)[:, None]
    n3g = np.arange(M2)[None, :] >> 7
    ag = np.arange(M2)[None, :] & 127
    t1 = np.exp(-2j * np.pi * (k1c * (ag * 16 + n3g)) / N)      # [k1, n3*128+a]
    n2c = np.arange(128)[:, None]
    k1g = np.arange(M2)[None, :] & 127
    it1 = np.conj(np.exp(-2j * np.pi * (k1g * (n2c * 16 + n3g)) / N))
    tabs = np.stack([t1.real, t1.imag, it1.real, it1.imag]).astype(np.float32)
    return IDX, mats_f32, tabs


_IDX, _MATS_F32, _TABS_F32 = _host_tables()
_NMATS = _MATS_F32.shape[0]


def _pq_tables(h):
    hp = np.zeros((2, 2, N))
    hp[:, :, :L] = h
    Hf = np.fft.fft(hp, axis=-1)
    G0 = Hf[0, 0] + 1j * Hf[1, 0]
    G1 = Hf[0, 1] + 1j * Hf[1, 1]
    Pt = (G0 - 1j * G1) / 2
    Qt = (G0 + 1j * G1) / 2
    Play = Pt[KMAP]
    Qtil = np.conj(Qt[KMAP][PPERM, :])
    return np.stack([Play.real, Play.imag, Qtil.real, Qtil.imag]).astype(np.float32)


# ---------------- bass program ----------------
_PROG = None


def _build_program():
    import concourse.bass as bass
    import concourse.tile as tile
    from concourse import bacc, mybir
    from contextlib import ExitStack

    f32 = mybir.dt.float32
    bf16 = mybir.dt.bfloat16
    nc = bacc.Bacc("TRN2", target_bir_lowering=False, debug=False,
                   enable_asserts=False, num_devices=NCORES)

    xp = nc.dram_tensor("xp", [2, 2, T], f32, kind="ExternalInput").ap()
    mats_d = nc.dram_tensor("mats", [_NMATS, 128, 128], bf16, kind="ExternalInput").ap()
    tabs_d = nc.dram_tensor("tabs", [4, 128, M2], bf16, kind="ExternalInput").ap()
    pq_d = nc.dram_tensor("pq", [4, 128, M2], bf16, kind="ExternalInput").ap()
    yp = nc.dram_tensor("yp", [2, 2, T], bf16, kind="ExternalOutput").ap()

    with tile.TileContext(nc) as tc, ExitStack() as ctx:
        cpool = ctx.enter_context(tc.tile_pool(name="consts", bufs=1))
        work = ctx.enter_context(tc.tile_pool(name="work", bufs=1))
        psA = ctx.enter_context(tc.tile_pool(name="psA", bufs=2, space="PSUM"))
        psB = ctx.enter_context(tc.tile_pool(name="psB", bufs=2, space="PSUM"))

        # ---- constants ----
        matst = cpool.tile([128, _NMATS * 128], bf16, tag="mats")
        tabst = cpool.tile([128, 4 * M2], bf16, tag="tabs")
        pqt = cpool.tile([128, 4 * M2], bf16, tag="pq")

        def _load_mats(eng, k0, k1):
            eng.dma_start(
                matst[:, k0 * 128:k1 * 128].rearrange("p (k c) -> p k c", k=k1 - k0),
                mats_d[k0:k1, :, :].rearrange("k p c -> p k c"))

        def _load_tab(eng, dstt, srct, k0, k1):
            eng.dma_start(
                dstt[:, k0 * M2:k1 * M2].rearrange("p (k c) -> p k c", k=k1 - k0),
                srct[k0:k1, :, :].rearrange("k p c -> p k c"))

        # s1 mats + t1 tabs now; the rest deferred into the schedule
        _load_mats(nc.sync, _IDX["s1"], _IDX["s1"] + 3)
        _load_tab(nc.scalar, tabst, tabs_d, 0, 2)          # t1
        deferred_loads = [
            (0.4, lambda: _load_mats(nc.sync, _IDX["s2f"], _IDX["s2f"] + 48)),
            (0.5, lambda: _load_mats(nc.scalar, _IDX["s3"], _IDX["s3"] + 3)),
            (4.5, lambda: _load_tab(nc.sync, pqt, pq_d, 0, 4)),
            (5.5, lambda: _load_mats(nc.scalar, _IDX["i1a"], _IDX["i1a"] + 6)),
            (6.5, lambda: _load_tab(nc.sync, tabst, tabs_d, 2, 4)),
            (7.5, lambda: _load_mats(nc.scalar, _IDX["s2i"], _IDX["s2i"] + 48)),
            (8.5, lambda: _load_mats(nc.sync, _IDX["s3i"], _IDX["s3i"] + 3)),
        ]

        def mat(i):
            return matst[:, i * 128:(i + 1) * 128]

        def m3(key, off=0):
            base = _IDX[key] + 3 * off
            return mat(base), mat(base + 1), mat(base + 2)

        def tab(i):
            return tabst[:, i * M2:(i + 1) * M2]

        t1r, t1i = tab(0), tab(1)
        it1r, it1i = tab(2), tab(3)
        Pr = pqt[:, 0:M2]
        Pi = pqt[:, M2:2 * M2]
        Qr = pqt[:, 2 * M2:3 * M2]
        Qi = pqt[:, 3 * M2:4 * M2]

        s1m = m3("s1")
        s2fm = [m3("s2f", n3) for n3 in range(16)]
        s3m = m3("s3")
        i1am = m3("i1a")
        i1bm = m3("i1b")
        s2im = [m3("s2i", n3) for n3 in range(16)]
        s3im = m3("s3i")

        # ---- helpers ----
        def _copy(eng, dst, src):
            if eng is nc.scalar:
                nc.scalar.copy(dst, src)
            else:
                eng.tensor_copy(dst, src)

        def copy_chunk(eng, dst, src_ps, ch, rearr):
            if rearr is None:
                _copy(eng, dst[:, ch * CW:(ch + 1) * CW], src_ps[:])
            elif rearr == "m2n":
                # psum cols m-local = (a-32ch)*16+n3 ; dst col n3*128+a
                psrc = src_ps[:].rearrange("p (a n3) -> p a n3", n3=16)
                ddst = dst[:].rearrange("p (n3 a) -> p a n3", n3=16)[
                    :, ch * 32:(ch + 1) * 32, :]
                _copy(eng, ddst, psrc)
            elif rearr == "k2n":
                psrc = src_ps[:].rearrange("p (k n3) -> p k n3", n3=16)
                ddst = dst[:].rearrange("p (n3 k) -> p k n3", n3=16)[
                    :, ch * 32:(ch + 1) * 32, :]
                _copy(eng, ddst, psrc)
            elif rearr == "n2m":
                # psum cols = (n3-4ch)*128+n2 ; dst col m = n2*16+n3
                psrc = src_ps[:].rearrange("p (n3 n2) -> p n2 n3", n3=4)
                ddst = dst[:].rearrange("p (n2 n3) -> p n2 n3", n3=16)[
                    :, :, ch * 4:(ch + 1) * 4]
                _copy(eng, ddst, psrc)

        def cstage_full(rhs_r, rhs_i, m3_, pspool, post):
            mr, mi, nmi = m3_
            for ch in range(NCH):
                sl = (slice(None), slice(ch * CW, (ch + 1) * CW))
                prr = pspool.tile([128, CW], f32, tag="pr")
                pii = pspool.tile([128, CW], f32, tag="pi")
                nc.tensor.matmul(prr[:], mr[:], rhs_r[sl], start=True, stop=False)
                nc.tensor.matmul(prr[:], nmi[:], rhs_i[sl], start=False, stop=True)
                nc.tensor.matmul(pii[:], mi[:], rhs_r[sl], start=True, stop=False)
                nc.tensor.matmul(pii[:], mr[:], rhs_i[sl], start=False, stop=True)
                post(prr, pii, ch)

        def s2_stage(rhs_r, rhs_i, mlist, pspool, post, rhs_kmajor):
            for ch in range(NCH):
                prr = pspool.tile([128, CW], f32, tag="pr")
                pii = pspool.tile([128, CW], f32, tag="pi")
                k0 = ch * 32
                for n3 in range(16):
                    mr, mi, nmi = mlist[n3]
                    osl = (slice(None), slice(n3, CW, 16))
                    if rhs_kmajor:
                        rsl = (slice(None),
                               slice(k0 * 16 + n3, (k0 + 31) * 16 + n3 + 1, 16))
                    else:
                        rsl = (slice(None),
                               slice(n3 * 128 + k0, n3 * 128 + k0 + 32))
                    nc.tensor.matmul(prr[osl], mr[:], rhs_r[rsl], start=True, stop=False)
                    nc.tensor.matmul(prr[osl], nmi[:], rhs_i[rsl], start=False, stop=True)
                    nc.tensor.matmul(pii[osl], mi[:], rhs_r[rsl], start=True, stop=False)
                    nc.tensor.matmul(pii[osl], mr[:], rhs_i[rsl], start=False, stop=True)
                post(prr, pii, ch)

        def cmul_chunk(dst_r, dst_i, ar, ai, br, bi, ch, lay,
                       mule1, mule2, addeng1, addeng2):
            """One CW-chunk of (dst_r + j dst_i) = (ar + j ai)*(br + j bi).
            lay: "c" contiguous chunk; "n3a" 3D region [p, n3(16), 32a]."""
            def reg(t_):
                if lay == "c":
                    return t_[:, ch * CW:(ch + 1) * CW]
                return t_[:].rearrange("p (n3 a) -> p n3 a", n3=16)[
                    :, :, ch * 32:(ch + 1) * 32]

            m0 = work.tile([128, CW], bf16, tag="cm0", bufs=8, name=f"m0c")
            m1 = work.tile([128, CW], bf16, tag="cm1", bufs=8, name=f"m1c")
            m2 = work.tile([128, CW], bf16, tag="cm2", bufs=8, name=f"m2c")
            m3_ = work.tile([128, CW], bf16, tag="cm3", bufs=8, name=f"m3c")
            arr, aii = reg(ar), reg(ai)
            brr, bii = reg(br), reg(bi)
            mule1.tensor_mul(m0[:].rearrange("p (n3 a) -> p n3 a", n3=16) if lay == "n3a" else m0[:], arr, brr)
            mule1.tensor_mul(m1[:].rearrange("p (n3 a) -> p n3 a", n3=16) if lay == "n3a" else m1[:], aii, bii)
            mule2.tensor_mul(m2[:].rearrange("p (n3 a) -> p n3 a", n3=16) if lay == "n3a" else m2[:], arr, bii)
            mule2.tensor_mul(m3_[:].rearrange("p (n3 a) -> p n3 a", n3=16) if lay == "n3a" else m3_[:], aii, brr)
            sm0 = m0[:].rearrange("p (n3 a) -> p n3 a", n3=16) if lay == "n3a" else m0[:]
            sm1 = m1[:].rearrange("p (n3 a) -> p n3 a", n3=16) if lay == "n3a" else m1[:]
            sm2 = m2[:].rearrange("p (n3 a) -> p n3 a", n3=16) if lay == "n3a" else m2[:]
            sm3 = m3_[:].rearrange("p (n3 a) -> p n3 a", n3=16) if lay == "n3a" else m3_[:]
            addeng1.tensor_sub(reg(dst_r), sm0, sm1)
            addeng2.tensor_add(reg(dst_i), sm2, sm3)

        def xbar_T(dst, src, eng):
            eng.dma_start_transpose(
                dst[:].rearrange("j (g p) -> j g p", p=128), src[:])

        # ---- load / store ----
        def load_block(in_r, in_i, b, blk):
            """Load x into bf16 tiles; only the nonzero partition range is
            populated (S1 contracts over ROWRNG[blk]); tiny edge memsets."""
            for pl, t_ in ((0, in_r), (1, in_i)):
                src = xp[b, pl]
                if blk == 0:
                    nc.vector.memset(t_[0:VROW + 1, :], 0.0)
                    nc.gpsimd.dma_start(
                        t_[VROW:VROW + 1, VCOL:M2],
                        src[0:ROW_TAIL].rearrange('(a b) -> a b', a=1))
                    nc.gpsimd.dma_start(
                        t_[VROW + 1:128, :],
                        src[ROW_TAIL:HOP].rearrange("(r m) -> r m", m=M2))
                elif blk == 1:
                    s0 = HOP - (L - 1)
                    nc.gpsimd.dma_start(
                        t_[:, :], src[s0:s0 + N].rearrange("(r m) -> r m", m=M2))
                else:
                    s0 = 2 * HOP - (L - 1)
                    nfull = (T - s0) // M2
                    rem = (T - s0) - nfull * M2
                    nc.vector.memset(t_[64:128, rem:M2], 0.0)
                    nc.gpsimd.dma_start(
                        t_[0:nfull, :],
                        src[s0:s0 + nfull * M2].rearrange("(r m) -> r m", m=M2))
                    nc.gpsimd.dma_start(
                        t_[nfull:nfull + 1, 0:rem],
                        src[s0 + nfull * M2:T].rearrange('(a b) -> a b', a=1))

        ROWRNG = {0: (0, 128), 1: (0, 128), 2: (0, 89)}

        def store_block(ysb0, ysb1, b, blk):
            for o, ysb in ((0, ysb0), (1, ysb1)):
                dst = yp[b, o]
                base = blk * HOP
                eng = nc.sync if o == 0 else nc.scalar
                eng.dma_start(
                    dst[base:base + ROW_TAIL].rearrange('(a b) -> a b', a=1),
                    ysb[VROW:VROW + 1, VCOL:M2])
                if blk < 2:
                    eng.dma_start(
                        dst[base + ROW_TAIL:base + HOP].rearrange("(r m) -> r m", m=M2),
                        ysb[VROW + 1:128, :])
                else:
                    nrem = T - base - ROW_TAIL
                    nfull = nrem // M2
                    rem = nrem - nfull * M2
                    eng.dma_start(
                        dst[base + ROW_TAIL:base + ROW_TAIL + nfull * M2]
                            .rearrange("(r m) -> r m", m=M2),
                        ysb[VROW + 1:VROW + 1 + nfull, :])
                    eng.dma_start(
                        dst[T - rem:T].rearrange('(a b) -> a b', a=1),
                        ysb[VROW + 1 + nfull:VROW + 2 + nfull, 0:rem])

        # ---- per-(batch, block) pipeline, stage closures ----
        def make_stages(b, blk, js):
            """Return list of stage closures for block (b, blk) using tag
            suffix js (job slot). Tiles are pre-created here; slot reuse:
            A: in->d2->z->sb->g2 ; B: g0->z2->w->s1c->h ; C: g->d3->sa->dp->d4."""
            sfx = f"_{js}"

            ctr = [0]

            def _slot(sl):
                ctr[0] += 1
                return (work.tile([128, M2], bf16, tag=sl + "r" + sfx,
                                  name=f"{sl}r{sfx}_{ctr[0]}"),
                        work.tile([128, M2], bf16, tag=sl + "i" + sfx,
                                  name=f"{sl}i{sfx}_{ctr[0]}"))

            def tA():
                return _slot("A")

            def tB():
                return _slot("B")

            def tC():
                return _slot("C")

            in_r, in_i = tA()
            g0r, g0i = tB()
            gr, gi = tC()
            d2r, d2i = tA()
            z2r, z2i = tB()
            d3r, d3i = tC()
            zr, zi = tA()
            wr_, wi_ = tB()
            sar, sai = tC()
            sbr, sbi = tA()
            s1cr, s1ci = tB()
            dpr, dpi = tC()
            g2r, g2i = tA()
            hr, hi = tB()
            d4r, d4i = tC()
            ysb0 = work.tile([128, M2], bf16, tag="ysb0" + sfx, name="ysb0" + sfx + str(blk))
            ysb1 = work.tile([128, M2], bf16, tag="ysb1" + sfx, name="ysb1" + sfx + str(blk))

            st = []
            st.append(lambda: load_block(in_r, in_i, b, blk))

            def f_s1_t1():
                mr, mi, nmi = s1m
                r0, r1 = ROWRNG[blk]
                for ch in range(NCH):
                    sl = (slice(r0, r1), slice(ch * CW, (ch + 1) * CW))
                    msl = slice(r0, r1)
                    prr = psA.tile([128, CW], f32, tag="pr")
                    pii = psA.tile([128, CW], f32, tag="pi")
                    nc.tensor.matmul(prr[:], mr[msl], in_r[sl], start=True, stop=False)
                    nc.tensor.matmul(prr[:], nmi[msl], in_i[sl], start=False, stop=True)
                    nc.tensor.matmul(pii[:], mi[msl], in_r[sl], start=True, stop=False)
                    nc.tensor.matmul(pii[:], mr[msl], in_i[sl], start=False, stop=True)
                    copy_chunk(nc.scalar, g0r, prr, ch, "m2n")
                    copy_chunk(nc.scalar, g0i, pii, ch, "m2n")
                    cmul_chunk(gr, gi, g0r, g0i, t1r, t1i, ch, "n3a",
                               nc.vector, nc.vector, nc.gpsimd, nc.vector)
            st.append(f_s1_t1)
            st.append(lambda: (xbar_T(d2r, gr, nc.sync),
                               xbar_T(d2i, gi, nc.sync)))
            st.append(lambda: s2_stage(
                d2r, d2i, s2fm, psA,
                lambda pr, pi, ch: (copy_chunk(nc.scalar, z2r, pr, ch, None),
                                    copy_chunk(nc.scalar, z2i, pi, ch, None)),
                rhs_kmajor=False))
            st.append(lambda: (xbar_T(d3r, z2r, nc.scalar),
                               xbar_T(d3i, z2i, nc.scalar)))
            st.append(lambda: cstage_full(
                d3r, d3i, s3m, psB,
                lambda pr, pi, ch: (copy_chunk(nc.scalar, zr, pr, ch, None),
                                    copy_chunk(nc.scalar, zi, pi, ch, None))))

            def w_stage():
                for zsrc, wdst in ((zr, wr_), (zi, wi_)):
                    nc.vector.tensor_copy(wdst[0:128, 0:M2], zsrc[0:128, M2 - 1::-1])
                    nc.vector.tensor_copy(wdst[0:16, 128:M2], zsrc[0:16, M2 - 1:127:-1])
                    nc.vector.tensor_copy(wdst[0:16, 1:128], zsrc[0:16, 127:0:-1])
                    nc.scalar.dma_start(wdst[0:15, 0:1], zsrc[1:16, 0:1])
                    nc.scalar.dma_start(wdst[15:16, 0:1], zsrc[0:1, 0:1])
            st.append(w_stage)

            def f_spec_s1inv():
                a0, a1, a2 = i1am
                b0, b1, b2 = i1bm
                for ch in range(NCH):
                    cmul_chunk(sar, sai, zr, zi, Pr, Pi, ch, "c",
                               nc.vector, nc.vector, nc.vector, nc.vector)
                    cmul_chunk(sbr, sbi, wr_, wi_, Qr, Qi, ch, "c",
                               nc.vector, nc.vector, nc.gpsimd, nc.gpsimd)
                    sl = (slice(None), slice(ch * CW, (ch + 1) * CW))
                    prr = psB.tile([128, CW], f32, tag="pr")
                    pii = psB.tile([128, CW], f32, tag="pi")
                    nc.tensor.matmul(prr[:], a0[:], sar[sl], start=True, stop=False)
                    nc.tensor.matmul(prr[:], a2[:], sai[sl], start=False, stop=False)
                    nc.tensor.matmul(prr[:], b0[:], sbr[sl], start=False, stop=False)
                    nc.tensor.matmul(prr[:], b1[:], sbi[sl], start=False, stop=True)
                    nc.tensor.matmul(pii[:], a1[:], sar[sl], start=True, stop=False)
                    nc.tensor.matmul(pii[:], a0[:], sai[sl], start=False, stop=False)
                    nc.tensor.matmul(pii[:], b1[:], sbr[sl], start=False, stop=False)
                    nc.tensor.matmul(pii[:], b2[:], sbi[sl], start=False, stop=True)
                    copy_chunk(nc.scalar, s1cr, prr, ch, None)
                    copy_chunk(nc.scalar, s1ci, pii, ch, None)
            st.append(f_spec_s1inv)
            st.append(lambda: (xbar_T(dpr, s1cr, nc.sync),
                               xbar_T(dpi, s1ci, nc.sync)))

            def f_s2i_it1():
                for ch in range(NCH):
                    prr = psB.tile([128, CW], f32, tag="pr")
                    pii = psB.tile([128, CW], f32, tag="pi")
                    k0 = ch * 32
                    for n3 in range(16):
                        mr, mi, nmi = s2im[n3]
                        osl = (slice(None), slice(n3, CW, 16))
                        rsl = (slice(None),
                               slice(k0 * 16 + n3, (k0 + 31) * 16 + n3 + 1, 16))
                        nc.tensor.matmul(prr[osl], mr[:], dpr[rsl], start=True, stop=False)
                        nc.tensor.matmul(prr[osl], nmi[:], dpi[rsl], start=False, stop=True)
                        nc.tensor.matmul(pii[osl], mi[:], dpr[rsl], start=True, stop=False)
                        nc.tensor.matmul(pii[osl], mr[:], dpi[rsl], start=False, stop=True)
                    copy_chunk(nc.scalar, g2r, prr, ch, "k2n")
                    copy_chunk(nc.scalar, g2i, pii, ch, "k2n")
                    cmul_chunk(hr, hi, g2r, g2i, it1r, it1i, ch, "n3a",
                               nc.vector, nc.vector, nc.gpsimd, nc.vector)
            st.append(f_s2i_it1)
            st.append(lambda: (xbar_T(d4r, hr, nc.sync),
                               xbar_T(d4i, hi, nc.sync)))
            st.append(lambda: cstage_full(
                d4r, d4i, s3im, psA,
                lambda pr, pi, ch: (copy_chunk(nc.scalar, ysb0, pr, ch, "n2m"),
                                    copy_chunk(nc.scalar, ysb1, pi, ch, "n2m"))))
            st.append(lambda: store_block(ysb0, ysb1, b, blk))

            return st

        jobs = [(0, 0), (1, 0), (0, 1), (1, 1), (0, 2), (1, 2)]
        WAVE2_OFF = 14
        chains = [make_stages(b_, blk_, j % 3) for j, (b_, blk_) in enumerate(jobs)]
        sched = []
        for j, ch in enumerate(chains):
            off = 0 if j < 3 else WAVE2_OFF
            for s, fn in enumerate(ch):
                sched.append((off + s, 10 + j, fn))
        for r, fn in deferred_loads:
            sched.append((r, 0, fn))
        sched.sort(key=lambda t: (t[0], t[1]))
        for _, _, fn in sched:
            fn()

    nc.compile()
    return nc


def _get_prog():
    global _PROG
    if _PROG is None:
        _PROG = _build_program()
    return _PROG


# ---------------- public entry ----------------
def kernel(x, b, c, U_raw, gamma_raw):
    from concourse import bass_utils

    x = np.ascontiguousarray(np.asarray(x, np.float32))
    h = _host_ir(np.asarray(b, np.float32), np.asarray(c, np.float32),
                 np.asarray(U_raw, np.float32), np.asarray(gamma_raw, np.float32))
    pqf = _pq_tables(h)
    mats_bf = _MATS_F32.astype(BFNP)
    tabs_bf = _TABS_F32.astype(BFNP)
    pq_bf = pqf.astype(BFNP)
    nc = _get_prog()

    in_maps = []
    for core in range(NCORES):
        in_maps.append({
            "xp": x[2 * core:2 * core + 2],
            "mats": mats_bf, "tabs": tabs_bf, "pq": pq_bf,
        })
    res = bass_utils.run_bass_kernel_spmd(nc, in_maps, core_ids=list(range(NCORES)))
    y = np.empty((16, 2, T), np.float32)
    for core in range(NCORES):
        y[2 * core:2 * core + 2] = res.results[core]["yp"].astype(np.float32)
    return y


# revision 20
# speedup vs baseline: 1.0531x; 1.0493x over previous
"""Trainium2 Bass kernel for nn_FDN_88012469830490 (optimized).

FDN reverb: IR synthesis on host (6x6 solves + FFT of the 2x2x88200 IR
into the device spectral layout); FFT convolution on device.

Per core (2 batches x 3 overlap-save blocks of N=262144 = 128*128*16):
  z = x0 + j*x1 channel packing; 3-stage forward FFT in bf16 matmuls with
  the stage-2 twiddle folded into 16 per-n3 stationary matrices; conjugate
  -reversed spectrum via reversed-stride copies with the partition
  permutation folded into the inverse first stage; y0 + j*y1 output
  packing -> single complex inverse FFT per block. Transposes are XBAR DMA
  block-transposes; twiddle/spectral cmuls in bf16 on DVE/GpSimd; psum
  evacuation on ACT/GpSimd/DVE; loads are casting SWDGE DMAs.
"""
import sys
import numpy as np

sys.path.insert(0, "/opt/trn_rl_repo")

import ml_dtypes

BFNP = ml_dtypes.bfloat16

# ---------------- problem constants ----------------
SR = 44100
DELAYS = np.array([997, 1153, 1327, 1559, 1801, 2099])
ND = 6
L = 88200
FB = L // 2 + 1
NDF = 49
T60 = 1.5
GAMMA_MAX = 10.0 ** ((-60.0 / SR / T60 * DELAYS) / 20.0)

T = 441000
N = 262144
P1, P2, P3 = 128, 128, 16
M2 = P2 * P3          # 2048
HOP = N - (L - 1)     # 173945
NBLK = 3
NCORES = 8

VROW, VCOL = 43, 135
ROW_TAIL = 2048 - VCOL            # 1913

NCH = 4
CW = 512


# ---------------- host IR synthesis ----------------
def _expm_skew(S):
    lam, V = np.linalg.eigh(1j * S)
    return (V @ np.diag(np.exp(-1j * lam)) @ V.conj().T).real


def _host_ir(b, c, U_raw, gamma_raw):
    tri = np.triu(U_raw.astype(np.float64), 1)
    U = _expm_skew(tri - tri.T)
    gamma = (1.0 / (1.0 + np.exp(-gamma_raw.astype(np.float64)))) * GAMMA_MAX
    pos = np.arange(FB) * ((NDF - 1) / (FB - 1))
    i0 = np.clip(np.floor(pos).astype(int), 0, NDF - 2)
    frac = (pos - i0)[:, None]
    g = gamma[i0] * (1 - frac) + gamma[i0 + 1] * frac
    A = U[None, :, :] * g[:, None, :]
    freqs = np.arange(FB) / L * 2 * np.pi
    invD = np.exp(1j * freqs[:, None] * DELAYS)
    Mm = invD[:, :, None] * np.eye(ND) - A
    bc = np.broadcast_to(b.astype(np.float64), (FB, ND, 2))
    X = np.linalg.solve(Mm, bc)
    H = np.einsum('ci,fio->fco', c.astype(complex), X)
    h = np.fft.irfft(H.transpose(1, 2, 0), n=L)             # (2, 2, L)
    return h


# ---------------- spectral layout + tables ----------------
def _kmap():
    P = np.arange(128)[:, None]
    F = np.arange(M2)[None, :]
    k1 = (P >> 4) + 8 * (F >> 7)
    return k1 + 128 * (F & 127) + 16384 * (P & 15)


KMAP = _kmap()
PPERM = np.array([((8 - (p >> 4)) % 8) * 16 + (15 - (p & 15)) for p in range(128)])


def _m3(M):
    """lhsT triple (Mr.T, Mi.T, -Mi.T) for out = M @ rhs (standard)."""
    Mt = M.T
    return [np.ascontiguousarray(Mt.real, np.float32),
            np.ascontiguousarray(Mt.imag, np.float32),
            np.ascontiguousarray(-Mt.imag, np.float32)]


def _m3c(M):
    """lhsT triple (Mr.T, Mi.T, -Mr.T) for out = M @ conj(rhs)."""
    Mt = M.T
    return [np.ascontiguousarray(Mt.real, np.float32),
            np.ascontiguousarray(Mt.imag, np.float32),
            np.ascontiguousarray(-Mt.real, np.float32)]


def _host_tables():
    F128 = np.exp(-2j * np.pi * np.outer(np.arange(128), np.arange(128)) / 128)
    F16 = np.exp(-2j * np.pi * np.outer(np.arange(16), np.arange(16)) / 16)
    Bc = np.zeros((128, 128), complex)
    for kk in range(8):
        Bc[kk * 16:(kk + 1) * 16, kk * 16:(kk + 1) * 16] = F16

    mats = []
    IDX = {}

    def push(tr, key=None):
        if key is not None:
            IDX[key] = len(mats)
        mats.extend(tr)

    push(_m3(F128), "s1")
    IDX["s2f"] = len(mats)
    for n3 in range(16):
        tw = np.exp(-2j * np.pi * np.arange(128) * n3 / M2)
        push(_m3(F128 * tw[:, None]))
    push(_m3(Bc), "s3")
    BH = np.conj(Bc)                   # = conj(Bc).T, Bc block-symmetric
    push(_m3(BH), "i1a")
    push(_m3c(BH[:, PPERM]), "i1b")
    IDX["s2i"] = len(mats)
    for n3 in range(16):
        tw = np.exp(-2j * np.pi * np.arange(128) * n3 / M2)
        M = (np.conj(F128) * np.conj(tw)[:, None]).T   # [n2, q1]
        push(_m3(M))
    push(_m3(np.conj(F128) / N), "s3i")
    mats_f32 = np.stack(mats)

    # wide twiddle tables, n3-major layouts
    k1c = np.arange(128)[:, None]
    n3g = np.arange(M2)[None, :] >> 7
    ag = np.arange(M2)[None, :] & 127
    t1 = np.exp(-2j * np.pi * (k1c * (ag * 16 + n3g)) / N)      # [k1, n3*128+a]
    n2c = np.arange(128)[:, None]
    k1g = np.arange(M2)[None, :] & 127
    it1 = np.conj(np.exp(-2j * np.pi * (k1g * (n2c * 16 + n3g)) / N))
    tabs = np.stack([t1.real, t1.imag, it1.real, it1.imag]).astype(np.float32)
    return IDX, mats_f32, tabs


_IDX, _MATS_F32, _TABS_F32 = _host_tables()
_NMATS = _MATS_F32.shape[0]


def _pq_tables(h):
    hp = np.zeros((2, 2, N))
    hp[:, :, :L] = h
    Hf = np.fft.fft(hp, axis=-1)
    G0 = Hf[0, 0] + 1j * Hf[1, 0]
    G1 = Hf[0, 1] + 1j * Hf[1, 1]
    Pt = (G0 - 1j * G1) / 2
    Qt = (G0 + 1j * G1) / 2
    Play = Pt[KMAP]
    Qtil = np.conj(Qt[KMAP][PPERM, :])
    return np.stack([Play.real, Play.imag, Qtil.real, Qtil.imag]).astype(np.float32)


# ---------------- bass program ----------------
_PROG = None


def _build_program():
    import concourse.bass as bass
    import concourse.tile as tile
    from concourse import bacc, mybir
    from contextlib import ExitStack

    f32 = mybir.dt.float32
    bf16 = mybir.dt.bfloat16
    nc = bacc.Bacc("TRN2", target_bir_lowering=False, debug=False,
                   enable_asserts=False, num_devices=NCORES)

    xp = nc.dram_tensor("xp", [2, 2, T], bf16, kind="ExternalInput").ap()
    mats_d = nc.dram_tensor("mats", [_NMATS, 128, 128], bf16, kind="ExternalInput").ap()
    tabs_d = nc.dram_tensor("tabs", [4, 128, M2], bf16, kind="ExternalInput").ap()
    pq_d = nc.dram_tensor("pq", [4, 128, M2], bf16, kind="ExternalInput").ap()
    yp = nc.dram_tensor("yp", [2, 2, T], bf16, kind="ExternalOutput").ap()

    with tile.TileContext(nc) as tc, ExitStack() as ctx:
        cpool = ctx.enter_context(tc.tile_pool(name="consts", bufs=1))
        work = ctx.enter_context(tc.tile_pool(name="work", bufs=1))
        psA = ctx.enter_context(tc.tile_pool(name="psA", bufs=2, space="PSUM"))
        psB = ctx.enter_context(tc.tile_pool(name="psB", bufs=2, space="PSUM"))

        # ---- constants ----
        matst = cpool.tile([128, _NMATS * 128], bf16, tag="mats")
        tabst = cpool.tile([128, 4 * M2], bf16, tag="tabs")
        pqt = cpool.tile([128, 4 * M2], bf16, tag="pq")

        def _load_mats(eng, k0, k1):
            eng.dma_start(
                matst[:, k0 * 128:k1 * 128].rearrange("p (k c) -> p k c", k=k1 - k0),
                mats_d[k0:k1, :, :].rearrange("k p c -> p k c"))

        def _load_tab(eng, dstt, srct, k0, k1):
            eng.dma_start(
                dstt[:, k0 * M2:k1 * M2].rearrange("p (k c) -> p k c", k=k1 - k0),
                srct[k0:k1, :, :].rearrange("k p c -> p k c"))

        # s1 mats + t1 tabs now; the rest deferred into the schedule
        _load_mats(nc.sync, _IDX["s1"], _IDX["s1"] + 3)
        _load_tab(nc.scalar, tabst, tabs_d, 0, 2)          # t1
        deferred_loads = [
            (0.4, lambda: _load_mats(nc.sync, _IDX["s2f"], _IDX["s2f"] + 48)),
            (0.5, lambda: _load_mats(nc.scalar, _IDX["s3"], _IDX["s3"] + 3)),
            (4.5, lambda: _load_tab(nc.sync, pqt, pq_d, 0, 4)),
            (5.5, lambda: _load_mats(nc.scalar, _IDX["i1a"], _IDX["i1a"] + 6)),
            (6.5, lambda: _load_tab(nc.sync, tabst, tabs_d, 2, 4)),
            (7.5, lambda: _load_mats(nc.scalar, _IDX["s2i"], _IDX["s2i"] + 48)),
            (8.5, lambda: _load_mats(nc.sync, _IDX["s3i"], _IDX["s3i"] + 3)),
        ]

        def mat(i):
            return matst[:, i * 128:(i + 1) * 128]

        def m3(key, off=0):
            base = _IDX[key] + 3 * off
            return mat(base), mat(base + 1), mat(base + 2)

        def tab(i):
            return tabst[:, i * M2:(i + 1) * M2]

        t1r, t1i = tab(0), tab(1)
        it1r, it1i = tab(2), tab(3)
        Pr = pqt[:, 0:M2]
        Pi = pqt[:, M2:2 * M2]
        Qr = pqt[:, 2 * M2:3 * M2]
        Qi = pqt[:, 3 * M2:4 * M2]

        s1m = m3("s1")
        s2fm = [m3("s2f", n3) for n3 in range(16)]
        s3m = m3("s3")
        i1am = m3("i1a")
        i1bm = m3("i1b")
        s2im = [m3("s2i", n3) for n3 in range(16)]
        s3im = m3("s3i")

        # ---- helpers ----
        def _copy(eng, dst, src):
            if eng is nc.scalar:
                nc.scalar.copy(dst, src)
            else:
                eng.tensor_copy(dst, src)

        def copy_chunk(eng, dst, src_ps, ch, rearr):
            if rearr is None:
                _copy(eng, dst[:, ch * CW:(ch + 1) * CW], src_ps[:])
            elif rearr == "m2n":
                # psum cols m-local = (a-32ch)*16+n3 ; dst col n3*128+a
                psrc = src_ps[:].rearrange("p (a n3) -> p a n3", n3=16)
                ddst = dst[:].rearrange("p (n3 a) -> p a n3", n3=16)[
                    :, ch * 32:(ch + 1) * 32, :]
                _copy(eng, ddst, psrc)
            elif rearr == "k2n":
                psrc = src_ps[:].rearrange("p (k n3) -> p k n3", n3=16)
                ddst = dst[:].rearrange("p (n3 k) -> p k n3", n3=16)[
                    :, ch * 32:(ch + 1) * 32, :]
                _copy(eng, ddst, psrc)
            elif rearr == "n2m":
                # psum cols = (n3-4ch)*128+n2 ; dst col m = n2*16+n3
                psrc = src_ps[:].rearrange("p (n3 n2) -> p n2 n3", n3=4)
                ddst = dst[:].rearrange("p (n2 n3) -> p n2 n3", n3=16)[
                    :, :, ch * 4:(ch + 1) * 4]
                _copy(eng, ddst, psrc)

        def cstage_full(rhs_r, rhs_i, m3_, pspool, post):
            mr, mi, nmi = m3_
            for ch in range(NCH):
                sl = (slice(None), slice(ch * CW, (ch + 1) * CW))
                prr = pspool.tile([128, CW], f32, tag="pr")
                pii = pspool.tile([128, CW], f32, tag="pi")
                nc.tensor.matmul(prr[:], mr[:], rhs_r[sl], start=True, stop=False)
                nc.tensor.matmul(prr[:], nmi[:], rhs_i[sl], start=False, stop=True)
                nc.tensor.matmul(pii[:], mi[:], rhs_r[sl], start=True, stop=False)
                nc.tensor.matmul(pii[:], mr[:], rhs_i[sl], start=False, stop=True)
                post(prr, pii, ch)

        def s2_stage(rhs_r, rhs_i, mlist, pspool, post, rhs_kmajor):
            for ch in range(NCH):
                prr = pspool.tile([128, CW], f32, tag="pr")
                pii = pspool.tile([128, CW], f32, tag="pi")
                k0 = ch * 32
                for n3 in range(16):
                    mr, mi, nmi = mlist[n3]
                    osl = (slice(None), slice(n3, CW, 16))
                    if rhs_kmajor:
                        rsl = (slice(None),
                               slice(k0 * 16 + n3, (k0 + 31) * 16 + n3 + 1, 16))
                    else:
                        rsl = (slice(None),
                               slice(n3 * 128 + k0, n3 * 128 + k0 + 32))
                    nc.tensor.matmul(prr[osl], mr[:], rhs_r[rsl], start=True, stop=False)
                    nc.tensor.matmul(prr[osl], nmi[:], rhs_i[rsl], start=False, stop=True)
                    nc.tensor.matmul(pii[osl], mi[:], rhs_r[rsl], start=True, stop=False)
                    nc.tensor.matmul(pii[osl], mr[:], rhs_i[rsl], start=False, stop=True)
                post(prr, pii, ch)

        def cmul_chunk(dst_r, dst_i, ar, ai, br, bi, ch, lay,
                       mule1, mule2, addeng1, addeng2):
            """One CW-chunk of (dst_r + j dst_i) = (ar + j ai)*(br + j bi).
            lay: "c" contiguous chunk; "n3a" 3D region [p, n3(16), 32a]."""
            def reg(t_):
                if lay == "c":
                    return t_[:, ch * CW:(ch + 1) * CW]
                return t_[:].rearrange("p (n3 a) -> p n3 a", n3=16)[
                    :, :, ch * 32:(ch + 1) * 32]

            m0 = work.tile([128, CW], bf16, tag="cm0", bufs=8, name=f"m0c")
            m1 = work.tile([128, CW], bf16, tag="cm1", bufs=8, name=f"m1c")
            m2 = work.tile([128, CW], bf16, tag="cm2", bufs=8, name=f"m2c")
            m3_ = work.tile([128, CW], bf16, tag="cm3", bufs=8, name=f"m3c")
            arr, aii = reg(ar), reg(ai)
            brr, bii = reg(br), reg(bi)
            mule1.tensor_mul(m0[:].rearrange("p (n3 a) -> p n3 a", n3=16) if lay == "n3a" else m0[:], arr, brr)
            mule1.tensor_mul(m1[:].rearrange("p (n3 a) -> p n3 a", n3=16) if lay == "n3a" else m1[:], aii, bii)
            mule2.tensor_mul(m2[:].rearrange("p (n3 a) -> p n3 a", n3=16) if lay == "n3a" else m2[:], arr, bii)
            mule2.tensor_mul(m3_[:].rearrange("p (n3 a) -> p n3 a", n3=16) if lay == "n3a" else m3_[:], aii, brr)
            sm0 = m0[:].rearrange("p (n3 a) -> p n3 a", n3=16) if lay == "n3a" else m0[:]
            sm1 = m1[:].rearrange("p (n3 a) -> p n3 a", n3=16) if lay == "n3a" else m1[:]
            sm2 = m2[:].rearrange("p (n3 a) -> p n3 a", n3=16) if lay == "n3a" else m2[:]
            sm3 = m3_[:].rearrange("p (n3 a) -> p n3 a", n3=16) if lay == "n3a" else m3_[:]
            addeng1.tensor_sub(reg(dst_r), sm0, sm1)
            addeng2.tensor_add(reg(dst_i), sm2, sm3)

        def xbar_T(dst, src, eng):
            eng.dma_start_transpose(
                dst[:].rearrange("j (g p) -> j g p", p=128), src[:])

        # ---- load / store ----
        def load_block(in_r, in_i, b, blk):
            """Load x into bf16 tiles; only the nonzero partition range is
            populated (S1 contracts over ROWRNG[blk]); tiny edge memsets."""
            for pl, t_ in ((0, in_r), (1, in_i)):
                src = xp[b, pl]
                ldeng = nc.sync if pl == 0 else nc.scalar
                if blk == 0:
                    nc.gpsimd.memset(t_[0:VROW + 1, :], 0.0)
                    ldeng.dma_start(
                        t_[VROW:VROW + 1, VCOL:M2],
                        src[0:ROW_TAIL].rearrange('(a b) -> a b', a=1))
                    ldeng.dma_start(
                        t_[VROW + 1:128, :],
                        src[ROW_TAIL:HOP].rearrange("(r m) -> r m", m=M2))
                elif blk == 1:
                    s0 = HOP - (L - 1)
                    ldeng.dma_start(
                        t_[:, :], src[s0:s0 + N].rearrange("(r m) -> r m", m=M2))
                else:
                    s0 = 2 * HOP - (L - 1)
                    nfull = (T - s0) // M2
                    rem = (T - s0) - nfull * M2
                    nc.gpsimd.memset(t_[64:128, rem:M2], 0.0)
                    ldeng.dma_start(
                        t_[0:nfull, :],
                        src[s0:s0 + nfull * M2].rearrange("(r m) -> r m", m=M2))
                    ldeng.dma_start(
                        t_[nfull:nfull + 1, 0:rem],
                        src[s0 + nfull * M2:T].rearrange('(a b) -> a b', a=1))

        ROWRNG = {0: (0, 128), 1: (0, 128), 2: (0, 89)}

        def store_block(ysb0, ysb1, b, blk):
            for o, ysb in ((0, ysb0), (1, ysb1)):
                dst = yp[b, o]
                base = blk * HOP
                eng = nc.sync if o == 0 else nc.scalar
                eng.dma_start(
                    dst[base:base + ROW_TAIL].rearrange('(a b) -> a b', a=1),
                    ysb[VROW:VROW + 1, VCOL:M2])
                if blk < 2:
                    eng.dma_start(
                        dst[base + ROW_TAIL:base + HOP].rearrange("(r m) -> r m", m=M2),
                        ysb[VROW + 1:128, :])
                else:
                    nrem = T - base - ROW_TAIL
                    nfull = nrem // M2
                    rem = nrem - nfull * M2
                    eng.dma_start(
                        dst[base + ROW_TAIL:base + ROW_TAIL + nfull * M2]
                            .rearrange("(r m) -> r m", m=M2),
                        ysb[VROW + 1:VROW + 1 + nfull, :])
                    eng.dma_start(
                        dst[T - rem:T].rearrange('(a b) -> a b', a=1),
                        ysb[VROW + 1 + nfull:VROW + 2 + nfull, 0:rem])

        # ---- per-(batch, block) pipeline, stage closures ----
        def make_stages(b, blk, js):
            """Return list of stage closures for block (b, blk) using tag
            suffix js (job slot). Tiles are pre-created here; slot reuse:
            A: in->d2->z->sb->g2 ; B: g0->z2->w->s1c->h ; C: g->d3->sa->dp->d4."""
            sfx = f"_{js}"

            ctr = [0]

            def _slot(sl):
                ctr[0] += 1
                return (work.tile([128, M2], bf16, tag=sl + "r" + sfx,
                                  name=f"{sl}r{sfx}_{ctr[0]}"),
                        work.tile([128, M2], bf16, tag=sl + "i" + sfx,
                                  name=f"{sl}i{sfx}_{ctr[0]}"))

            def tA():
                return _slot("A")

            def tB():
                return _slot("B")

            def tC():
                return _slot("C")

            in_r, in_i = tA()
            g0r, g0i = tB()
            gr, gi = tC()
            d2r, d2i = tA()
            z2r, z2i = tB()
            d3r, d3i = tC()
            zr, zi = tA()
            wr_, wi_ = tB()
            sar, sai = tC()
            sbr, sbi = tA()
            s1cr, s1ci = tB()
            dpr, dpi = tC()
            g2r, g2i = tA()
            hr, hi = tB()
            d4r, d4i = tC()
            ysb0 = work.tile([128, M2], bf16, tag="ysb0" + sfx, name="ysb0" + sfx + str(blk))
            ysb1 = work.tile([128, M2], bf16, tag="ysb1" + sfx, name="ysb1" + sfx + str(blk))

            st = []
            st.append(lambda: load_block(in_r, in_i, b, blk))

            def f_s1_t1():
                mr, mi, nmi = s1m
                r0, r1 = ROWRNG[blk]
                for ch in range(NCH):
                    sl = (slice(r0, r1), slice(ch * CW, (ch + 1) * CW))
                    msl = slice(r0, r1)
                    prr = psA.tile([128, CW], f32, tag="pr")
                    pii = psA.tile([128, CW], f32, tag="pi")
                    nc.tensor.matmul(prr[:], mr[msl], in_r[sl], start=True, stop=False)
                    nc.tensor.matmul(prr[:], nmi[msl], in_i[sl], start=False, stop=True)
                    nc.tensor.matmul(pii[:], mi[msl], in_r[sl], start=True, stop=False)
                    nc.tensor.matmul(pii[:], mr[msl], in_i[sl], start=False, stop=True)
                    copy_chunk(nc.scalar, g0r, prr, ch, "m2n")
                    copy_chunk(nc.scalar, g0i, pii, ch, "m2n")
                    cmul_chunk(gr, gi, g0r, g0i, t1r, t1i, ch, "n3a",
                               nc.vector, nc.vector, nc.gpsimd, nc.vector)
            st.append(f_s1_t1)
            st.append(lambda: (xbar_T(d2r, gr, nc.sync),
                               xbar_T(d2i, gi, nc.sync)))
            st.append(lambda: s2_stage(
                d2r, d2i, s2fm, psA,
                lambda pr, pi, ch: (copy_chunk(nc.scalar, z2r, pr, ch, None),
                                    copy_chunk(nc.scalar, z2i, pi, ch, None)),
                rhs_kmajor=False))
            st.append(lambda: (xbar_T(d3r, z2r, nc.scalar),
                               xbar_T(d3i, z2i, nc.scalar)))
            st.append(lambda: cstage_full(
                d3r, d3i, s3m, psB,
                lambda pr, pi, ch: (copy_chunk(nc.scalar, zr, pr, ch, None),
                                    copy_chunk(nc.scalar, zi, pi, ch, None))))

            def w_stage():
                for zsrc, wdst in ((zr, wr_), (zi, wi_)):
                    nc.vector.tensor_copy(wdst[0:128, 0:M2], zsrc[0:128, M2 - 1::-1])
                    nc.vector.tensor_copy(wdst[0:16, 128:M2], zsrc[0:16, M2 - 1:127:-1])
                    nc.vector.tensor_copy(wdst[0:16, 1:128], zsrc[0:16, 127:0:-1])
                    nc.scalar.dma_start(wdst[0:15, 0:1], zsrc[1:16, 0:1])
                    nc.scalar.dma_start(wdst[15:16, 0:1], zsrc[0:1, 0:1])
            st.append(w_stage)

            def f_spec_s1inv():
                a0, a1, a2 = i1am
                b0, b1, b2 = i1bm
                for ch in range(NCH):
                    cmul_chunk(sar, sai, zr, zi, Pr, Pi, ch, "c",
                               nc.vector, nc.vector, nc.vector, nc.vector)
                    cmul_chunk(sbr, sbi, wr_, wi_, Qr, Qi, ch, "c",
                               nc.vector, nc.vector, nc.gpsimd, nc.gpsimd)
                    sl = (slice(None), slice(ch * CW, (ch + 1) * CW))
                    prr = psB.tile([128, CW], f32, tag="pr")
                    pii = psB.tile([128, CW], f32, tag="pi")
                    nc.tensor.matmul(prr[:], a0[:], sar[sl], start=True, stop=False)
                    nc.tensor.matmul(prr[:], a2[:], sai[sl], start=False, stop=False)
                    nc.tensor.matmul(prr[:], b0[:], sbr[sl], start=False, stop=False)
                    nc.tensor.matmul(prr[:], b1[:], sbi[sl], start=False, stop=True)
                    nc.tensor.matmul(pii[:], a1[:], sar[sl], start=True, stop=False)
                    nc.tensor.matmul(pii[:], a0[:], sai[sl], start=False, stop=False)
                    nc.tensor.matmul(pii[:], b1[:], sbr[sl], start=False, stop=False)
                    nc.tensor.matmul(pii[:], b2[:], sbi[sl], start=False, stop=True)
                    copy_chunk(nc.scalar, s1cr, prr, ch, None)
                    copy_chunk(nc.scalar, s1ci, pii, ch, None)
            st.append(f_spec_s1inv)
            st.append(lambda: (xbar_T(dpr, s1cr, nc.sync),
                               xbar_T(dpi, s1ci, nc.sync)))

            def f_s2i_it1():
                for ch in range(NCH):
                    prr = psB.tile([128, CW], f32, tag="pr")
                    pii = psB.tile([128, CW], f32, tag="pi")
                    k0 = ch * 32
                    for n3 in range(16):
                        mr, mi, nmi = s2im[n3]
                        osl = (slice(None), slice(n3, CW, 16))
                        rsl = (slice(None),
                               slice(k0 * 16 + n3, (k0 + 31) * 16 + n3 + 1, 16))
                        nc.tensor.matmul(prr[osl], mr[:], dpr[rsl], start=True, stop=False)
                        nc.tensor.matmul(prr[osl], nmi[:], dpi[rsl], start=False, stop=True)
                        nc.tensor.matmul(pii[osl], mi[:], dpr[rsl], start=True, stop=False)
                        nc.tensor.matmul(pii[osl], mr[:], dpi[rsl], start=False, stop=True)
                    copy_chunk(nc.scalar, g2r, prr, ch, "k2n")
                    copy_chunk(nc.scalar, g2i, pii, ch, "k2n")
                    cmul_chunk(hr, hi, g2r, g2i, it1r, it1i, ch, "n3a",
                               nc.vector, nc.vector, nc.gpsimd, nc.vector)
            st.append(f_s2i_it1)
            st.append(lambda: (xbar_T(d4r, hr, nc.sync),
                               xbar_T(d4i, hi, nc.sync)))
            st.append(lambda: cstage_full(
                d4r, d4i, s3im, psA,
                lambda pr, pi, ch: (copy_chunk(nc.scalar, ysb0, pr, ch, "n2m"),
                                    copy_chunk(nc.scalar, ysb1, pi, ch, "n2m"))))
            st.append(lambda: store_block(ysb0, ysb1, b, blk))

            return st

        jobs = [(0, 0), (1, 0), (0, 1), (1, 1), (0, 2), (1, 2)]
        WAVE2_OFF = 14
        chains = [make_stages(b_, blk_, j % 3) for j, (b_, blk_) in enumerate(jobs)]
        sched = []
        for j, ch in enumerate(chains):
            off = 0 if j < 3 else WAVE2_OFF
            for s, fn in enumerate(ch):
                sched.append((off + s, 10 + j, fn))
        for r, fn in deferred_loads:
            sched.append((r, 0, fn))
        sched.sort(key=lambda t: (t[0], t[1]))
        for _, _, fn in sched:
            fn()

    nc.compile()
    return nc


def _get_prog():
    global _PROG
    if _PROG is None:
        _PROG = _build_program()
    return _PROG


# ---------------- public entry ----------------
def kernel(x, b, c, U_raw, gamma_raw):
    from concourse import bass_utils

    x = np.ascontiguousarray(np.asarray(x, np.float32).astype(BFNP))
    h = _host_ir(np.asarray(b, np.float32), np.asarray(c, np.float32),
                 np.asarray(U_raw, np.float32), np.asarray(gamma_raw, np.float32))
    pqf = _pq_tables(h)
    mats_bf = _MATS_F32.astype(BFNP)
    tabs_bf = _TABS_F32.astype(BFNP)
    pq_bf = pqf.astype(BFNP)
    nc = _get_prog()

    in_maps = []
    for core in range(NCORES):
        in_maps.append({
            "xp": x[2 * core:2 * core + 2],
            "mats": mats_bf, "tabs": tabs_bf, "pq": pq_bf,
        })
    res = bass_utils.run_bass_kernel_spmd(nc, in_maps, core_ids=list(range(NCORES)))
    y = np.empty((16, 2, T), np.float32)
    for core in range(NCORES):
        y[2 * core:2 * core + 2] = res.results[core]["yp"].astype(np.float32)
    return y


# revision 28
# speedup vs baseline: 1.0560x; 1.0027x over previous
"""Trainium2 Bass kernel for nn_FDN_88012469830490 (optimized).

FDN reverb: IR synthesis on host (6x6 solves + FFT of the 2x2x88200 IR
into the device spectral layout); FFT convolution on device.

Per core (2 batches x 3 overlap-save blocks of N=262144 = 128*128*16):
  z = x0 + j*x1 channel packing; 3-stage forward FFT in bf16 matmuls with
  the stage-2 twiddle folded into 16 per-n3 stationary matrices; conjugate
  -reversed spectrum via reversed-stride copies with the partition
  permutation folded into the inverse first stage; y0 + j*y1 output
  packing -> single complex inverse FFT per block. Transposes are XBAR DMA
  block-transposes; twiddle/spectral cmuls in bf16 on DVE/GpSimd; psum
  evacuation on ACT/GpSimd/DVE; loads are casting SWDGE DMAs.
"""
import sys
import numpy as np

sys.path.insert(0, "/opt/trn_rl_repo")

import ml_dtypes

BFNP = ml_dtypes.bfloat16

# ---------------- problem constants ----------------
SR = 44100
DELAYS = np.array([997, 1153, 1327, 1559, 1801, 2099])
ND = 6
L = 88200
FB = L // 2 + 1
NDF = 49
T60 = 1.5
GAMMA_MAX = 10.0 ** ((-60.0 / SR / T60 * DELAYS) / 20.0)

T = 441000
N = 262144
P1, P2, P3 = 128, 128, 16
M2 = P2 * P3          # 2048
HOP = N - (L - 1)     # 173945
NBLK = 3
NCORES = 8

VROW, VCOL = 43, 135
ROW_TAIL = 2048 - VCOL            # 1913

NCH = 4
CW = 512


# ---------------- host IR synthesis ----------------
def _expm_skew(S):
    lam, V = np.linalg.eigh(1j * S)
    return (V @ np.diag(np.exp(-1j * lam)) @ V.conj().T).real


def _host_ir(b, c, U_raw, gamma_raw):
    tri = np.triu(U_raw.astype(np.float64), 1)
    U = _expm_skew(tri - tri.T)
    gamma = (1.0 / (1.0 + np.exp(-gamma_raw.astype(np.float64)))) * GAMMA_MAX
    pos = np.arange(FB) * ((NDF - 1) / (FB - 1))
    i0 = np.clip(np.floor(pos).astype(int), 0, NDF - 2)
    frac = (pos - i0)[:, None]
    g = gamma[i0] * (1 - frac) + gamma[i0 + 1] * frac
    A = U[None, :, :] * g[:, None, :]
    freqs = np.arange(FB) / L * 2 * np.pi
    invD = np.exp(1j * freqs[:, None] * DELAYS)
    Mm = invD[:, :, None] * np.eye(ND) - A
    bc = np.broadcast_to(b.astype(np.float64), (FB, ND, 2))
    X = np.linalg.solve(Mm, bc)
    H = np.einsum('ci,fio->fco', c.astype(complex), X)
    h = np.fft.irfft(H.transpose(1, 2, 0), n=L)             # (2, 2, L)
    return h


# ---------------- spectral layout + tables ----------------
def _kmap():
    P = np.arange(128)[:, None]
    F = np.arange(M2)[None, :]
    k1 = (P >> 4) + 8 * (F >> 7)
    return k1 + 128 * (F & 127) + 16384 * (P & 15)


KMAP = _kmap()
PPERM = np.array([((8 - (p >> 4)) % 8) * 16 + (15 - (p & 15)) for p in range(128)])


def _m3(M):
    """lhsT triple (Mr.T, Mi.T, -Mi.T) for out = M @ rhs (standard)."""
    Mt = M.T
    return [np.ascontiguousarray(Mt.real, np.float32),
            np.ascontiguousarray(Mt.imag, np.float32),
            np.ascontiguousarray(-Mt.imag, np.float32)]


def _m3c(M):
    """lhsT triple (Mr.T, Mi.T, -Mr.T) for out = M @ conj(rhs)."""
    Mt = M.T
    return [np.ascontiguousarray(Mt.real, np.float32),
            np.ascontiguousarray(Mt.imag, np.float32),
            np.ascontiguousarray(-Mt.real, np.float32)]


def _host_tables():
    F128 = np.exp(-2j * np.pi * np.outer(np.arange(128), np.arange(128)) / 128)
    F16 = np.exp(-2j * np.pi * np.outer(np.arange(16), np.arange(16)) / 16)
    Bc = np.zeros((128, 128), complex)
    for kk in range(8):
        Bc[kk * 16:(kk + 1) * 16, kk * 16:(kk + 1) * 16] = F16

    mats = []
    IDX = {}

    def push(tr, key=None):
        if key is not None:
            IDX[key] = len(mats)
        mats.extend(tr)

    push(_m3(F128), "s1")
    IDX["s2f"] = len(mats)
    for n3 in range(16):
        tw = np.exp(-2j * np.pi * np.arange(128) * n3 / M2)
        push(_m3(F128 * tw[:, None]))
    push(_m3(Bc), "s3")
    BH = np.conj(Bc)                   # = conj(Bc).T, Bc block-symmetric
    push(_m3(BH), "i1a")
    push(_m3c(BH[:, PPERM]), "i1b")
    IDX["s2i"] = len(mats)
    for n3 in range(16):
        tw = np.exp(-2j * np.pi * np.arange(128) * n3 / M2)
        M = (np.conj(F128) * np.conj(tw)[:, None]).T   # [n2, q1]
        push(_m3(M))
    push(_m3(np.conj(F128) / N), "s3i")
    mats_f32 = np.stack(mats)

    # wide twiddle tables, n3-major layouts
    k1c = np.arange(128)[:, None]
    n3g = np.arange(M2)[None, :] >> 7
    ag = np.arange(M2)[None, :] & 127
    t1 = np.exp(-2j * np.pi * (k1c * (ag * 16 + n3g)) / N)      # [k1, n3*128+a]
    n2c = np.arange(128)[:, None]
    k1g = np.arange(M2)[None, :] & 127
    it1 = np.conj(np.exp(-2j * np.pi * (k1g * (n2c * 16 + n3g)) / N))
    tabs = np.stack([t1.real, t1.imag, it1.real, it1.imag]).astype(np.float32)
    return IDX, mats_f32, tabs


_IDX, _MATS_F32, _TABS_F32 = _host_tables()
_NMATS = _MATS_F32.shape[0]


def _pq_tables(h):
    hp = np.zeros((2, 2, N))
    hp[:, :, :L] = h
    Hf = np.fft.fft(hp, axis=-1)
    G0 = Hf[0, 0] + 1j * Hf[1, 0]
    G1 = Hf[0, 1] + 1j * Hf[1, 1]
    Pt = (G0 - 1j * G1) / 2
    Qt = (G0 + 1j * G1) / 2
    Play = Pt[KMAP]
    Qtil = np.conj(Qt[KMAP][PPERM, :])
    return np.stack([Play.real, Play.imag, Qtil.real, Qtil.imag]).astype(np.float32)


# ---------------- bass program ----------------
_PROG = None


def _build_program():
    import concourse.bass as bass
    import concourse.tile as tile
    from concourse import bacc, mybir
    from contextlib import ExitStack

    f32 = mybir.dt.float32
    bf16 = mybir.dt.bfloat16
    nc = bacc.Bacc("TRN2", target_bir_lowering=False, debug=False,
                   enable_asserts=False, num_devices=NCORES)

    xp = nc.dram_tensor("xp", [2, 2, T], bf16, kind="ExternalInput").ap()
    mats_d = nc.dram_tensor("mats", [_NMATS, 128, 128], bf16, kind="ExternalInput").ap()
    tabs_d = nc.dram_tensor("tabs", [4, 128, M2], bf16, kind="ExternalInput").ap()
    pq_d = nc.dram_tensor("pq", [4, 128, M2], bf16, kind="ExternalInput").ap()
    yp = nc.dram_tensor("yp", [2, 2, T], bf16, kind="ExternalOutput").ap()

    with tile.TileContext(nc) as tc, ExitStack() as ctx:
        cpool = ctx.enter_context(tc.tile_pool(name="consts", bufs=1))
        work = ctx.enter_context(tc.tile_pool(name="work", bufs=1))
        psA = ctx.enter_context(tc.tile_pool(name="psA", bufs=2, space="PSUM"))
        psB = ctx.enter_context(tc.tile_pool(name="psB", bufs=2, space="PSUM"))

        # ---- constants ----
        matst = cpool.tile([128, _NMATS * 128], bf16, tag="mats")
        tabst = cpool.tile([128, 4 * M2], bf16, tag="tabs")
        pqt = cpool.tile([128, 4 * M2], bf16, tag="pq")

        def _load_mats(eng, k0, k1):
            eng.dma_start(
                matst[:, k0 * 128:k1 * 128].rearrange("p (k c) -> p k c", k=k1 - k0),
                mats_d[k0:k1, :, :].rearrange("k p c -> p k c"))

        def _load_tab(eng, dstt, srct, k0, k1):
            eng.dma_start(
                dstt[:, k0 * M2:k1 * M2].rearrange("p (k c) -> p k c", k=k1 - k0),
                srct[k0:k1, :, :].rearrange("k p c -> p k c"))

        # s1 mats + t1 tabs now; the rest deferred into the schedule
        _load_mats(nc.sync, _IDX["s1"], _IDX["s1"] + 3)
        _load_tab(nc.scalar, tabst, tabs_d, 0, 2)          # t1
        deferred_loads = [
            (0.4, lambda: _load_mats(nc.sync, _IDX["s2f"], _IDX["s2f"] + 48)),
            (0.5, lambda: _load_mats(nc.scalar, _IDX["s3"], _IDX["s3"] + 3)),
            (4.5, lambda: _load_tab(nc.sync, pqt, pq_d, 0, 4)),
            (5.5, lambda: _load_mats(nc.scalar, _IDX["i1a"], _IDX["i1a"] + 6)),
            (6.5, lambda: _load_tab(nc.sync, tabst, tabs_d, 2, 4)),
            (7.5, lambda: _load_mats(nc.scalar, _IDX["s2i"], _IDX["s2i"] + 48)),
            (8.5, lambda: _load_mats(nc.sync, _IDX["s3i"], _IDX["s3i"] + 3)),
        ]

        def mat(i):
            return matst[:, i * 128:(i + 1) * 128]

        def m3(key, off=0):
            base = _IDX[key] + 3 * off
            return mat(base), mat(base + 1), mat(base + 2)

        def tab(i):
            return tabst[:, i * M2:(i + 1) * M2]

        t1r, t1i = tab(0), tab(1)
        it1r, it1i = tab(2), tab(3)
        Pr = pqt[:, 0:M2]
        Pi = pqt[:, M2:2 * M2]
        Qr = pqt[:, 2 * M2:3 * M2]
        Qi = pqt[:, 3 * M2:4 * M2]

        s1m = m3("s1")
        s2fm = [m3("s2f", n3) for n3 in range(16)]
        s3m = m3("s3")
        i1am = m3("i1a")
        i1bm = m3("i1b")
        s2im = [m3("s2i", n3) for n3 in range(16)]
        s3im = m3("s3i")

        # ---- helpers ----
        def _copy(eng, dst, src):
            if eng is nc.scalar:
                nc.scalar.copy(dst, src)
            else:
                eng.tensor_copy(dst, src)

        def copy_chunk(eng, dst, src_ps, ch, rearr):
            if rearr is None:
                _copy(eng, dst[:, ch * CW:(ch + 1) * CW], src_ps[:])
            elif rearr == "m2n":
                # psum cols m-local = (a-32ch)*16+n3 ; dst col n3*128+a
                psrc = src_ps[:].rearrange("p (a n3) -> p a n3", n3=16)
                ddst = dst[:].rearrange("p (n3 a) -> p a n3", n3=16)[
                    :, ch * 32:(ch + 1) * 32, :]
                _copy(eng, ddst, psrc)
            elif rearr == "k2n":
                psrc = src_ps[:].rearrange("p (k n3) -> p k n3", n3=16)
                ddst = dst[:].rearrange("p (n3 k) -> p k n3", n3=16)[
                    :, ch * 32:(ch + 1) * 32, :]
                _copy(eng, ddst, psrc)
            elif rearr == "n2m":
                # psum cols = (n3-4ch)*128+n2 ; dst col m = n2*16+n3
                psrc = src_ps[:].rearrange("p (n3 n2) -> p n2 n3", n3=4)
                ddst = dst[:].rearrange("p (n2 n3) -> p n2 n3", n3=16)[
                    :, :, ch * 4:(ch + 1) * 4]
                _copy(eng, ddst, psrc)

        def cstage_full(rhs_r, rhs_i, m3_, pspool, post):
            mr, mi, nmi = m3_
            for ch in range(NCH):
                sl = (slice(None), slice(ch * CW, (ch + 1) * CW))
                prr = pspool.tile([128, CW], f32, tag="pr")
                pii = pspool.tile([128, CW], f32, tag="pi")
                nc.tensor.matmul(prr[:], mr[:], rhs_r[sl], start=True, stop=False)
                nc.tensor.matmul(prr[:], nmi[:], rhs_i[sl], start=False, stop=True)
                nc.tensor.matmul(pii[:], mi[:], rhs_r[sl], start=True, stop=False)
                nc.tensor.matmul(pii[:], mr[:], rhs_i[sl], start=False, stop=True)
                post(prr, pii, ch)

        def s2_stage(rhs_r, rhs_i, mlist, pspool, post, rhs_kmajor):
            for ch in range(NCH):
                prr = pspool.tile([128, CW], f32, tag="pr")
                pii = pspool.tile([128, CW], f32, tag="pi")
                k0 = ch * 32
                for n3 in range(16):
                    mr, mi, nmi = mlist[n3]
                    osl = (slice(None), slice(n3, CW, 16))
                    if rhs_kmajor:
                        rsl = (slice(None),
                               slice(k0 * 16 + n3, (k0 + 31) * 16 + n3 + 1, 16))
                    else:
                        rsl = (slice(None),
                               slice(n3 * 128 + k0, n3 * 128 + k0 + 32))
                    nc.tensor.matmul(prr[osl], mr[:], rhs_r[rsl], start=True, stop=False)
                    nc.tensor.matmul(prr[osl], nmi[:], rhs_i[rsl], start=False, stop=True)
                    nc.tensor.matmul(pii[osl], mi[:], rhs_r[rsl], start=True, stop=False)
                    nc.tensor.matmul(pii[osl], mr[:], rhs_i[rsl], start=False, stop=True)
                post(prr, pii, ch)

        def cmul_chunk(dst_r, dst_i, ar, ai, br, bi, ch, lay,
                       mule1, mule2, addeng1, addeng2):
            """One CW-chunk of (dst_r + j dst_i) = (ar + j ai)*(br + j bi).
            lay: "c" contiguous chunk; "n3a" 3D region [p, n3(16), 32a]."""
            def reg(t_):
                if lay == "c":
                    return t_[:, ch * CW:(ch + 1) * CW]
                return t_[:].rearrange("p (n3 a) -> p n3 a", n3=16)[
                    :, :, ch * 32:(ch + 1) * 32]

            m0 = work.tile([128, CW], bf16, tag="cm0", bufs=8, name=f"m0c")
            m1 = work.tile([128, CW], bf16, tag="cm1", bufs=8, name=f"m1c")
            m2 = work.tile([128, CW], bf16, tag="cm2", bufs=8, name=f"m2c")
            m3_ = work.tile([128, CW], bf16, tag="cm3", bufs=8, name=f"m3c")
            arr, aii = reg(ar), reg(ai)
            brr, bii = reg(br), reg(bi)
            mule1.tensor_mul(m0[:].rearrange("p (n3 a) -> p n3 a", n3=16) if lay == "n3a" else m0[:], arr, brr)
            mule1.tensor_mul(m1[:].rearrange("p (n3 a) -> p n3 a", n3=16) if lay == "n3a" else m1[:], aii, bii)
            mule2.tensor_mul(m2[:].rearrange("p (n3 a) -> p n3 a", n3=16) if lay == "n3a" else m2[:], arr, bii)
            mule2.tensor_mul(m3_[:].rearrange("p (n3 a) -> p n3 a", n3=16) if lay == "n3a" else m3_[:], aii, brr)
            sm0 = m0[:].rearrange("p (n3 a) -> p n3 a", n3=16) if lay == "n3a" else m0[:]
            sm1 = m1[:].rearrange("p (n3 a) -> p n3 a", n3=16) if lay == "n3a" else m1[:]
            sm2 = m2[:].rearrange("p (n3 a) -> p n3 a", n3=16) if lay == "n3a" else m2[:]
            sm3 = m3_[:].rearrange("p (n3 a) -> p n3 a", n3=16) if lay == "n3a" else m3_[:]
            addeng1.tensor_sub(reg(dst_r), sm0, sm1)
            addeng2.tensor_add(reg(dst_i), sm2, sm3)

        def xbar_T(dst, src, eng):
            eng.dma_start_transpose(
                dst[:].rearrange("j (g p) -> j g p", p=128), src[:])

        # ---- load / store ----
        def load_block(in_r, in_i, b, blk):
            """Load x into bf16 tiles; only the nonzero partition range is
            populated (S1 contracts over ROWRNG[blk]); tiny edge memsets."""
            for pl, t_ in ((0, in_r), (1, in_i)):
                src = xp[b, pl]
                ldeng = nc.sync if pl == 0 else nc.scalar
                if blk == 0:
                    nc.gpsimd.memset(t_[0:VROW + 1, :], 0.0)
                    ldeng.dma_start(
                        t_[VROW:VROW + 1, VCOL:M2],
                        src[0:ROW_TAIL].rearrange('(a b) -> a b', a=1))
                    ldeng.dma_start(
                        t_[VROW + 1:128, :],
                        src[ROW_TAIL:HOP].rearrange("(r m) -> r m", m=M2))
                elif blk == 1:
                    s0 = HOP - (L - 1)
                    ldeng.dma_start(
                        t_[:, :], src[s0:s0 + N].rearrange("(r m) -> r m", m=M2))
                else:
                    s0 = 2 * HOP - (L - 1)
                    nfull = (T - s0) // M2
                    rem = (T - s0) - nfull * M2
                    nc.gpsimd.memset(t_[64:128, rem:M2], 0.0)
                    ldeng.dma_start(
                        t_[0:nfull, :],
                        src[s0:s0 + nfull * M2].rearrange("(r m) -> r m", m=M2))
                    ldeng.dma_start(
                        t_[nfull:nfull + 1, 0:rem],
                        src[s0 + nfull * M2:T].rearrange('(a b) -> a b', a=1))

        ROWRNG = {0: (0, 128), 1: (0, 128), 2: (0, 89)}

        def store_block(ysb0, ysb1, b, blk):
            for o, ysb in ((0, ysb0), (1, ysb1)):
                dst = yp[b, o]
                base = blk * HOP
                eng = nc.sync if o == 0 else nc.scalar
                eng.dma_start(
                    dst[base:base + ROW_TAIL].rearrange('(a b) -> a b', a=1),
                    ysb[VROW:VROW + 1, VCOL:M2])
                if blk < 2:
                    eng.dma_start(
                        dst[base + ROW_TAIL:base + HOP].rearrange("(r m) -> r m", m=M2),
                        ysb[VROW + 1:128, :])
                else:
                    nrem = T - base - ROW_TAIL
                    nfull = nrem // M2
                    rem = nrem - nfull * M2
                    eng.dma_start(
                        dst[base + ROW_TAIL:base + ROW_TAIL + nfull * M2]
                            .rearrange("(r m) -> r m", m=M2),
                        ysb[VROW + 1:VROW + 1 + nfull, :])
                    eng.dma_start(
                        dst[T - rem:T].rearrange('(a b) -> a b', a=1),
                        ysb[VROW + 1 + nfull:VROW + 2 + nfull, 0:rem])

        # ---- per-(batch, block) pipeline, stage closures ----
        def make_stages(b, blk, js):
            """Return list of stage closures for block (b, blk) using tag
            suffix js (job slot). Tiles are pre-created here; slot reuse:
            A: in->d2->z->sb->g2 ; B: g0->z2->w->s1c->h ; C: g->d3->sa->dp->d4."""
            sfx = f"_{js}"

            ctr = [0]

            def _slot(sl):
                ctr[0] += 1
                return (work.tile([128, M2], bf16, tag=sl + "r" + sfx,
                                  name=f"{sl}r{sfx}_{ctr[0]}"),
                        work.tile([128, M2], bf16, tag=sl + "i" + sfx,
                                  name=f"{sl}i{sfx}_{ctr[0]}"))

            def tA():
                return _slot("A")

            def tB():
                return _slot("B")

            def tC():
                return _slot("C")

            in_r, in_i = tA()
            g0r, g0i = tB()
            gr, gi = tC()
            d2r, d2i = tA()
            z2r, z2i = tB()
            d3r, d3i = tC()
            zr, zi = tA()
            wr_, wi_ = tB()
            sar, sai = tC()
            sbr, sbi = tA()
            s1cr, s1ci = tB()
            dpr, dpi = tC()
            g2r, g2i = tA()
            hr, hi = tB()
            d4r, d4i = tC()
            ysb0 = work.tile([128, M2], bf16, tag="ysb0" + sfx, name="ysb0" + sfx + str(blk))
            ysb1 = work.tile([128, M2], bf16, tag="ysb1" + sfx, name="ysb1" + sfx + str(blk))

            st = []
            st.append(lambda: load_block(in_r, in_i, b, blk))

            def f_s1_t1():
                mr, mi, nmi = s1m
                r0, r1 = ROWRNG[blk]
                for ch in range(NCH):
                    sl = (slice(r0, r1), slice(ch * CW, (ch + 1) * CW))
                    msl = slice(r0, r1)
                    prr = psA.tile([128, CW], f32, tag="pr")
                    pii = psA.tile([128, CW], f32, tag="pi")
                    nc.tensor.matmul(prr[:], mr[msl], in_r[sl], start=True, stop=False)
                    nc.tensor.matmul(prr[:], nmi[msl], in_i[sl], start=False, stop=True)
                    nc.tensor.matmul(pii[:], mi[msl], in_r[sl], start=True, stop=False)
                    nc.tensor.matmul(pii[:], mr[msl], in_i[sl], start=False, stop=True)
                    copy_chunk(nc.scalar, g0r, prr, ch, "m2n")
                    copy_chunk(nc.scalar, g0i, pii, ch, "m2n")
                    cmul_chunk(gr, gi, g0r, g0i, t1r, t1i, ch, "n3a",
                               nc.vector, nc.vector, nc.gpsimd, nc.vector)
            st.append(f_s1_t1)
            st.append(lambda: (xbar_T(d2r, gr, nc.sync),
                               xbar_T(d2i, gi, nc.sync)))
            st.append(lambda: s2_stage(
                d2r, d2i, s2fm, psA,
                lambda pr, pi, ch: (copy_chunk(nc.scalar, z2r, pr, ch, None),
                                    copy_chunk(nc.scalar, z2i, pi, ch, None)),
                rhs_kmajor=False))
            st.append(lambda: (xbar_T(d3r, z2r, nc.scalar),
                               xbar_T(d3i, z2i, nc.scalar)))
            st.append(lambda: cstage_full(
                d3r, d3i, s3m, psB,
                lambda pr, pi, ch: (copy_chunk(nc.scalar, zr, pr, ch, None),
                                    copy_chunk(nc.scalar, zi, pi, ch, None))))

            def w_stage():
                for zsrc, wdst in ((zr, wr_), (zi, wi_)):
                    nc.vector.tensor_copy(wdst[0:128, 0:M2], zsrc[0:128, M2 - 1::-1])
                    nc.vector.tensor_copy(wdst[0:16, 128:M2], zsrc[0:16, M2 - 1:127:-1])
                    nc.vector.tensor_copy(wdst[0:16, 1:128], zsrc[0:16, 127:0:-1])
                    nc.scalar.dma_start(wdst[0:15, 0:1], zsrc[1:16, 0:1])
                    nc.scalar.dma_start(wdst[15:16, 0:1], zsrc[0:1, 0:1])
            st.append(w_stage)

            def f_spec_s1inv():
                a0, a1, a2 = i1am
                b0, b1, b2 = i1bm
                for ch in range(NCH):
                    cmul_chunk(sar, sai, zr, zi, Pr, Pi, ch, "c",
                               nc.vector, nc.vector, nc.vector, nc.vector)
                    sl = (slice(None), slice(ch * CW, (ch + 1) * CW))
                    prr = psB.tile([128, CW], f32, tag="pr")
                    pii = psB.tile([128, CW], f32, tag="pi")
                    nc.tensor.matmul(prr[:], a0[:], sar[sl], start=True, stop=False)
                    nc.tensor.matmul(prr[:], a2[:], sai[sl], start=False, stop=False)
                    nc.tensor.matmul(pii[:], a1[:], sar[sl], start=True, stop=False)
                    nc.tensor.matmul(pii[:], a0[:], sai[sl], start=False, stop=False)
                    cmul_chunk(sbr, sbi, wr_, wi_, Qr, Qi, ch, "c",
                               nc.vector, nc.vector, nc.gpsimd, nc.gpsimd)
                    nc.tensor.matmul(prr[:], b0[:], sbr[sl], start=False, stop=False)
                    nc.tensor.matmul(prr[:], b1[:], sbi[sl], start=False, stop=True)
                    nc.tensor.matmul(pii[:], b1[:], sbr[sl], start=False, stop=False)
                    nc.tensor.matmul(pii[:], b2[:], sbi[sl], start=False, stop=True)
                    copy_chunk(nc.scalar, s1cr, prr, ch, None)
                    copy_chunk(nc.scalar, s1ci, pii, ch, None)
            st.append(f_spec_s1inv)
            st.append(lambda: (xbar_T(dpr, s1cr, nc.sync),
                               xbar_T(dpi, s1ci, nc.sync)))

            def f_s2i_it1():
                for ch in range(NCH):
                    prr = psB.tile([128, CW], f32, tag="pr")
                    pii = psB.tile([128, CW], f32, tag="pi")
                    k0 = ch * 32
                    for n3 in range(16):
                        mr, mi, nmi = s2im[n3]
                        osl = (slice(None), slice(n3, CW, 16))
                        rsl = (slice(None),
                               slice(k0 * 16 + n3, (k0 + 31) * 16 + n3 + 1, 16))
                        nc.tensor.matmul(prr[osl], mr[:], dpr[rsl], start=True, stop=False)
                        nc.tensor.matmul(prr[osl], nmi[:], dpi[rsl], start=False, stop=True)
                        nc.tensor.matmul(pii[osl], mi[:], dpr[rsl], start=True, stop=False)
                        nc.tensor.matmul(pii[osl], mr[:], dpi[rsl], start=False, stop=True)
                    copy_chunk(nc.scalar, g2r, prr, ch, "k2n")
                    copy_chunk(nc.scalar, g2i, pii, ch, "k2n")
                    cmul_chunk(hr, hi, g2r, g2i, it1r, it1i, ch, "n3a",
                               nc.vector, nc.vector, nc.gpsimd, nc.vector)
            st.append(f_s2i_it1)
            st.append(lambda: (xbar_T(d4r, hr, nc.sync),
                               xbar_T(d4i, hi, nc.sync)))
            st.append(lambda: cstage_full(
                d4r, d4i, s3im, psA,
                lambda pr, pi, ch: (copy_chunk(nc.scalar, ysb0, pr, ch, "n2m"),
                                    copy_chunk(nc.scalar, ysb1, pi, ch, "n2m"))))
            st.append(lambda: store_block(ysb0, ysb1, b, blk))

            return st

        jobs = [(0, 0), (1, 0), (0, 1), (1, 1), (0, 2), (1, 2)]
        WAVE2_OFF = 14
        chains = [make_stages(b_, blk_, j % 3) for j, (b_, blk_) in enumerate(jobs)]
        sched = []
        for j, ch in enumerate(chains):
            off = 0 if j < 3 else WAVE2_OFF
            for s, fn in enumerate(ch):
                sched.append((off + s, 10 + j, fn))
        for r, fn in deferred_loads:
            sched.append((r, 0, fn))
        sched.sort(key=lambda t: (t[0], t[1]))
        for _, _, fn in sched:
            fn()

    nc.compile()
    return nc


def _get_prog():
    global _PROG
    if _PROG is None:
        _PROG = _build_program()
    return _PROG


# ---------------- public entry ----------------
def kernel(x, b, c, U_raw, gamma_raw):
    from concourse import bass_utils

    x = np.ascontiguousarray(np.asarray(x, np.float32).astype(BFNP))
    h = _host_ir(np.asarray(b, np.float32), np.asarray(c, np.float32),
                 np.asarray(U_raw, np.float32), np.asarray(gamma_raw, np.float32))
    pqf = _pq_tables(h)
    mats_bf = _MATS_F32.astype(BFNP)
    tabs_bf = _TABS_F32.astype(BFNP)
    pq_bf = pqf.astype(BFNP)
    nc = _get_prog()

    in_maps = []
    for core in range(NCORES):
        in_maps.append({
            "xp": x[2 * core:2 * core + 2],
            "mats": mats_bf, "tabs": tabs_bf, "pq": pq_bf,
        })
    res = bass_utils.run_bass_kernel_spmd(nc, in_maps, core_ids=list(range(NCORES)))
    y = np.empty((16, 2, T), np.float32)
    for core in range(NCORES):
        y[2 * core:2 * core + 2] = res.results[core]["yp"].astype(np.float32)
    return y
